# revision 59
# baseline (speedup 1.0000x reference)
"""Trainium2 Bass kernel for nn_Attention_86698209837214.

Multi-head attention: out = softmax(q k^T / 8) v @ W_out + b_out with
B=4, N=2048, DIM=1024, H=16, Dh=64.

Sharding: 8 cores = (batch b in 0..3) x (head-half hh in 0..1); each core
computes 8 heads of one batch. Host pre-transposes x[b], slices weights,
converts everything to bf16; host sums the two per-core partial outputs
per batch and adds b_out.

All matmul operands are bf16 (PSUM accumulation fp32). bf16 halves input
DMA and - critically - enables the PE fast-weight-load path that fp32r
(FP32-HIGH mode) disables, so LDWEIGHTS hides behind matmul streaming.
Measured end-to-end rel err vs the fp32 reference: ~5e-3 (gate 2e-2).

Every matmul in the kernel is a uniform [128,128]x[128,512] bf16 shape so
the PE never switches tile config: kT is stored zero-padded per head slot
(the other head's 64 partitions zeroed; dots stream the full qT slice and
the zero weights kill the cross terms) and v slots are padded to 128 cols.
Steady-state matmul spacing measured 215 ns (213 ns streaming floor).

DMA: inputs are host-packed so per-partition rows are 4-8 KB contiguous
runs (the DMA dispatcher is packet-rate bound: 1-2 KB packets move at
~20-40 GB/s/queue vs full rate at 8 KB), split across the two HWDGE
queues (SP + ACT). Output rows are spread across SP/gpsimd/ACT queues so
no single queue's backlog serializes the kernel tail.

Device dataflow per core:
  1. v = x @ Wv into v_aug tiles ([64 v | 1 ones | 63 zero] per slot);
     the ones column makes attn@v also produce the softmax denominator
     (row 64 of the av accumulator).
  2. kT (zero-padded per head slot), qT = (x @ Wk/Wq)^T in [c, n] layout.
  3. Attention with ic (i-chunk of 512) outer, hp (head pair) inner:
     dots^T per (s, j-tile); exp on ScalarE (scale=1/8 folded, no max
     subtraction - logits ~N(0,1)) writing bf16; attn@v lags dots by one
     j-pair so the PE never waits on a fresh exp. ScalarE exp is the
     co-critical engine (~278 us busy); an early chunk (hp0, ic0) runs
     during the qT projections to start it ~30 us sooner.
     Epilogue: denominator row -> partition 0 via DMA hop, fast
     reciprocal, gpsimd partition_broadcast, DVE multiply -> aT bf16.
  4. Out-projection with K=512 accumulated over all four head pairs in
     PSUM, so the core emits one full [2048,1024] fp32 partial (host
     adds the two per-batch cores + b_out). Each ic's eight PSUM groups
     are woven into the next ic's first chunk as fillers (the single
     out-proj PSUM bank's turnaround hides behind dots/av streaming);
     the last ic holds 4 groups back to fill the final epilogue window.

Known run-to-run variance: the chip's engine clocks flip between a fast
and a ~1.2x slower DVFS state per run (matmul median 379 vs 454 ns);
measured spans 428 us (fast) / ~506 us (slow) vs 619 us baseline.
"""

import sys

for _p in ("/opt/trn_rl_repo",):
    if _p not in sys.path:
        sys.path.append(_p)

from contextlib import ExitStack

import numpy as np
import ml_dtypes

import concourse.bass as bass  # noqa: F401
import concourse.tile as tile
from concourse import bacc, mybir
from concourse.bass_utils import run_bass_kernel_spmd

F32 = mybir.dt.float32
BF16 = mybir.dt.bfloat16
AF = mybir.ActivationFunctionType

P = 128
NSEQ = 2048  # sequence length per batch
D = 1024  # model dim
CH = 512  # per-core head-dim width (8 heads x 64)
DH = 64
NPAIR = 4  # head pairs per core (c-tiles of 128)
NDT = D // P  # 8 d-tiles
NNT = NSEQ // P  # 16 n-tiles
NNC = NSEQ // 512  # 4 n-chunks
NJP = NNT // 2  # 8 j-tile pairs
SCALE = 0.125  # DIM_HEAD ** -0.5


def build_program():
    nc = bacc.Bacc("TRN2", target_bir_lowering=False, debug=False)

    # Host-packed layouts: per-partition rows are large contiguous runs so
    # DMA packets are 4-8 KB (the dispatcher is packet-rate bound; 1-2 KB
    # packets measured ~20-40 GB/s/queue vs full rate at 8 KB).
    # xt_p[p, pc, dt, n'] = x^T[dt*128+p, pc*256+n']  (8 pieces of 256)
    xt_p = nc.dram_tensor("xt", [P, 8 * NDT * 256], BF16, kind="ExternalInput")
    # wqkv_p[p, proj, dt, c'] = W_proj[dt*128+p, c']  (proj: 0=q, 1=k, 2=v)
    wqkv_p = nc.dram_tensor("wqkv", [P, 3 * NDT * 512], BF16, kind="ExternalInput")
    # wout_p[p, ct, e] = W_out[ct*128+p, e]
    wout_p = nc.dram_tensor("wout", [P, NPAIR * D], BF16, kind="ExternalInput")
    out = nc.dram_tensor("out", [NSEQ, D], F32, kind="ExternalOutput")

    out_t = out.ap().rearrange("(nt p) e -> nt p e", p=P)  # [16, 128, 1024]

    with tile.TileContext(nc) as tc, ExitStack() as ctx:
        # ---- persistent pools ----
        p_qk = ctx.enter_context(tc.tile_pool(name="p_qk", bufs=1))  # 32 KB/p
        p_v = ctx.enter_context(tc.tile_pool(name="p_v", bufs=1))  # ~16 KB/p
        p_small = ctx.enter_context(tc.tile_pool(name="p_small", bufs=1))
        # PSUM: dots 2x[128,1024] (4 banks) + av 3x[65,512] (3) + out (1) = 8
        ps_mm = ctx.enter_context(tc.tile_pool(name="ps_mm", bufs=2, space="PSUM"))
        ps_av = ctx.enter_context(tc.tile_pool(name="ps_av", bufs=3, space="PSUM"))
        ps_out = ctx.enter_context(tc.tile_pool(name="ps_out", bufs=1, space="PSUM"))

        # attention-phase persistent pools (created up front: pool pop order
        # must be LIFO w.r.t. the temporary phase-A/B pools below)
        p_wout = ctx.enter_context(tc.tile_pool(name="p_wout", bufs=1))
        p_exp = ctx.enter_context(tc.tile_pool(name="p_exp", bufs=10))  # 20 KB/p
        p_aT = ctx.enter_context(tc.tile_pool(name="p_aT", bufs=8))
        p_den = ctx.enter_context(tc.tile_pool(name="p_den", bufs=1))
        p_recip = ctx.enter_context(tc.tile_pool(name="p_recip", bufs=1))
        p_bcast = ctx.enter_context(tc.tile_pool(name="p_bcast", bufs=3))
        p_ostage = ctx.enter_context(tc.tile_pool(name="p_ostage", bufs=2))

        # dummy exp: pulls the ~2.7us ACT_TABLE_LOAD for the Exp set into the
        # initial DMA wait instead of the first real softmax tile
        warm_in = p_small.tile([P, 1], BF16, tag="warm_in")
        nc.gpsimd.memset(warm_in, 1.0)
        warm = p_small.tile([P, 1], F32, tag="warm")
        nc.scalar.activation(out=warm, in_=warm_in, func=AF.Exp, scale=1.0)

        # ---- phase A: load weights + xt; compute v_aug ----
        # Input DMAs split across the two HWDGE queues (SP + ACT; ACT is
        # otherwise idle until the early attention chunk) in arrival order
        # of first use: v needs (wv, xt nc0..3), then kT needs wk, qT wq.
        # Pool stacks pushed xt,wq,wk,wv so they pop LIFO as phases finish.
        st_xt = ExitStack()
        p_xt = st_xt.enter_context(tc.tile_pool(name="p_xt", bufs=1))  # 32 KB/p
        st_wq = ExitStack()
        p_wq = st_wq.enter_context(tc.tile_pool(name="p_wq", bufs=1))  # 8 KB/p
        st_wk = ExitStack()
        p_wk = st_wk.enter_context(tc.tile_pool(name="p_wk", bufs=1))  # 8 KB/p
        st_wv = ExitStack()
        p_wv = st_wv.enter_context(tc.tile_pool(name="p_wv", bufs=1))  # 8 KB/p

        xt_sb = p_xt.tile([P, 8 * NDT * 256], BF16, tag="xt")
        w_sbs = [
            p_wq.tile([P, NDT * 512], BF16, tag="wq", name="wq"),
            p_wk.tile([P, NDT * 512], BF16, tag="wk", name="wk"),
            p_wv.tile([P, NDT * 512], BF16, tag="wv", name="wv"),
        ]
        xt_pieces = xt_sb.rearrange("p (pc r) -> p pc r", r=NDT * 256)
        xt_view = xt_sb.rearrange("p (pc dt n) -> p pc dt n", dt=NDT, n=256)
        xt_dram = xt_p.ap().rearrange("p (pc r) -> p pc r", r=NDT * 256)
        w_dram = wqkv_p.ap().rearrange("p (pr r) -> p pr r", r=NDT * 512)

        # wv in two dt-halves so v's first accumulation isn't gated on 1 MB
        hw = NDT * 256
        nc.scalar.dma_start(out=w_sbs[2][:, 0:hw], in_=w_dram[:, 2][:, 0:hw])
        for pc in range(8):
            q = nc.sync if pc % 2 == 0 else nc.scalar
            q.dma_start(out=xt_pieces[:, pc], in_=xt_dram[:, pc])
            if pc == 0:
                nc.scalar.dma_start(
                    out=w_sbs[2][:, hw : 2 * hw], in_=w_dram[:, 2][:, hw : 2 * hw]
                )
        nc.sync.dma_start(out=w_sbs[1], in_=w_dram[:, 1])  # wk
        nc.scalar.dma_start(out=w_sbs[0], in_=w_dram[:, 0])  # wq

        wout_sb = p_wout.tile([P, NPAIR * D], BF16, tag="wout")
        nc.sync.dma_start(out=wout_sb, in_=wout_p.ap())
        wout_tiles = [wout_sb[:, ct * D : (ct + 1) * D] for ct in range(NPAIR)]

        def xt_sl(dt, n0, w):
            pc, off = divmod(n0, 256)
            if off + w <= 256:
                base = (pc * NDT + dt) * 256 + off
                return xt_sb[:, base : base + w]
            assert off == 0 and w % 256 == 0
            return xt_view[:, pc : pc + w // 256, dt, :]

        def w_sl(proj, dt, c0, w):
            base = dt * 512 + c0
            return w_sbs[proj][:, base : base + w]

        # v_aug: per head-slot sg, 128 cols = [v_sg (64) | ones (1) | 0 (63)].
        # The ones column makes attn@v also produce the softmax denominator;
        # the zero pad keeps every stationary a full 128-column weight so the
        # PE never switches tile config and FWL stays eligible.
        v_tiles = []
        for nt in range(NNT):
            dst = p_v.tile([P, 8 * P], BF16, tag=f"v{nt}")
            pad = dst.rearrange("p (h c) -> p h c", c=P)[:, :, DH:P]
            nc.gpsimd.memset(pad, 0.0)
            ones_dst = dst.rearrange("p (h c) -> p h c", c=P)[:, :, DH : DH + 1]
            nc.gpsimd.memset(ones_dst, 1.0)
            v_tiles.append(dst)
        for nt in range(NNT):
            dst = v_tiles[nt]
            acc = ps_mm.tile([P, 512], F32, tag="mm")
            for dt_i in range(NDT):
                nc.tensor.matmul(
                    acc,
                    xt_sl(dt_i, nt * P, P),
                    w_sl(2, dt_i, 0, 512),
                    start=(dt_i == 0),
                    stop=(dt_i == NDT - 1),
                )
            v_dst = dst.rearrange("p (h c) -> p h c", c=P)[:, :, 0:DH]
            nc.vector.tensor_copy(v_dst, acc.rearrange("p (h c) -> p h c", c=DH))
        st_wv.close()

        # ---- phase B: kT (zero-padded per head-slot), then qT ----
        # kT is stored per head-slot s as [128, 2048] with the other head's
        # 64 partitions zeroed, so dots matmuls are full [128,128]x[128,512]
        # (moving = the full qT slice; zero weights kill the cross terms).
        kT_pad = []
        for ct in range(NPAIR):
            pair = []
            for s in range(2):
                t = p_qk.tile([P, NSEQ], BF16, tag=f"kp{ct}{s}", name=f"kp{ct}{s}")
                z0, z1 = (DH, P) if s == 0 else (0, DH)
                # gpsimd, not DVE: DVE must keep pace with the qk PSUM copies
                nc.gpsimd.memset(t[z0:z1, :], 0.0)
                pair.append(t)
            kT_pad.append(pair)
        qT_tiles = []

        def emit_qk_tile(which, proj, ct):
            woff = ct * P
            if which == "q":
                dst = p_qk.tile([P, NSEQ], BF16, tag=f"qT{ct}", name=f"qT{ct}")
            for nch in range(NNC):
                acc = ps_mm.tile([P, 512], F32, tag="mm", name="acc")
                for dt_i in range(NDT):
                    nc.tensor.matmul(
                        acc,
                        w_sl(proj, dt_i, woff, P),
                        xt_sl(dt_i, nch * 512, 512),
                        start=(dt_i == 0),
                        stop=(dt_i == NDT - 1),
                    )
                sl = slice(nch * 512, (nch + 1) * 512)
                if which == "k":
                    nc.vector.tensor_copy(kT_pad[ct][0][0:DH, sl], acc[0:DH, :])
                    nc.vector.tensor_copy(kT_pad[ct][1][DH:P, sl], acc[DH:P, :])
                else:
                    nc.vector.tensor_copy(dst[:, sl], acc)
            if which == "q":
                qT_tiles.append(dst)

        for ct in range(NPAIR):
            emit_qk_tile("k", 1, ct)
        st_wk.close()
        emit_qk_tile("q", 0, 0)

        # per (hp, ic) attention body -------------------------------------
        def emit_dots_av(hp, ic, av_ps, filler=None):
            """dots + exp + attn@v for one (head pair, i-chunk).

            filler(jp), if given, is invoked once per j-pair to weave in
            independent PE work (the previous chunk's out-projection) so
            PSUM-bank turnarounds hide behind dots/av streaming.
            """
            i0 = ic * 512

            def emit_av(jp, exp_pair):
                for s in range(2):
                    sg = hp * 2 + s
                    for half in range(2):
                        jtx = 2 * jp + half
                        nc.tensor.matmul(
                            av_ps[s],
                            v_tiles[jtx][:, sg * P : (sg + 1) * P],
                            exp_pair[s][:, half * 512 : (half + 1) * 512],
                            start=(jp == 0 and half == 0),
                            stop=(jp == NJP - 1 and half == 1),
                        )

            prev_exp = None
            for jp in range(NJP):
                # dots for 2 j-tiles x 2 head-slots; stationary = zero-padded
                # per-head kT block, moving = full qT slice
                dots_t = [
                    ps_mm.tile([P, 1024], F32, tag="mm", name=f"dots{s}")
                    for s in range(2)
                ]
                for half in range(2):
                    jtx = 2 * jp + half
                    for s in range(2):
                        nc.tensor.matmul(
                            dots_t[s][:, half * 512 : (half + 1) * 512],
                            kT_pad[hp][s][:, jtx * P : (jtx + 1) * P],
                            qT_tiles[hp][:, i0 : i0 + 512],
                            start=True,
                            stop=True,
                        )
                if prev_exp is not None:
                    emit_av(jp - 1, prev_exp)
                if filler is not None:
                    filler(jp)
                exp_tiles = []
                for s in range(2):
                    e = p_exp.tile([P, 1024], BF16, tag="exp")
                    nc.scalar.activation(
                        out=e, in_=dots_t[s], func=AF.Exp, scale=SCALE
                    )
                    exp_tiles.append(e)
                prev_exp = exp_tiles
            emit_av(NJP - 1, prev_exp)

        def emit_epilogue(hp, ic, av_ps, aT):
            """normalize: rows 0:64 = unnormalized attn-out, row 64 = denom."""
            den_hi = p_den.tile([65, 1024], F32, tag="den_hi")
            for s in range(2):
                nc.vector.tensor_copy(
                    den_hi[64:65, s * 512 : (s + 1) * 512], av_ps[s][64:65, :]
                )
            den_sb = p_den.tile([1, 1024], F32, tag="den_sb")
            nc.gpsimd.dma_start(out=den_sb, in_=den_hi[64:65, :])
            recip = p_recip.tile([1, 1024], F32, tag="recip")
            nc.vector.reciprocal_approx_fast(out=recip, in_=den_sb)
            bcast = []
            for s in range(2):
                bc = p_bcast.tile([DH, 512], F32, tag="bcast", name=f"bc{s}")
                nc.gpsimd.partition_broadcast(
                    out_ap=bc, in_ap=recip[:, s * 512 : (s + 1) * 512]
                )
                bcast.append(bc)
            nc.vector.tensor_mul(aT[0:DH, :], av_ps[0][0:DH, :], bcast[0])
            tmp = p_bcast.tile([DH, 512], BF16, tag="tmp")
            nc.vector.tensor_mul(tmp, av_ps[1][0:DH, :], bcast[1])
            nc.gpsimd.dma_start(out=aT[DH:P, :], in_=tmp)

        # ---- early chunk (hp=0, ic=0): ScalarE gets exp work while the
        # remaining qT tiles occupy the PE. Epilogue deferred to main loop.
        # early_av[0] lives in the (still idle) out-proj bank so the main
        # loop's first av pair doesn't WAR-wait on this chunk's epilogue
        early_av = [
            ps_out.tile([P, 512], F32, tag="o", name="eav0"),
            ps_av.tile([P, 512], F32, tag="av", name="eav1"),
        ]
        emit_dots_av(0, 0, early_av)

        for ct in range(1, NPAIR):
            emit_qk_tile("q", 0, ct)
        st_wq.close()
        st_xt.close()

        # ---- main loop: ic outer, hp inner; out-projection per ic with
        # K=512 accumulated over all four head pairs. Each ic's eight
        # out-projection groups are spread as fillers over ALL FOUR of the
        # next ic's chunks (2 per chunk) so per-chunk PE work stays balanced
        # against the fixed 16 exps/chunk on ScalarE, and the single
        # out-proj PSUM bank's turnaround hides behind dots/av streaming.
        def make_groups(aT_tiles, ic):
            state = {}
            groups = []
            for g in range(NJP):
                nt = 4 * ic + g // 2
                ntl = g // 2
                ec = g % 2

                def group(pool=None, act_copy=False, nt=nt, ntl=ntl, ec=ec):
                    pl = pool if pool is not None else ps_out
                    o_ps = pl.tile(
                        [P, 512], F32, tag="o" if pl is ps_out else "mm", name="ops"
                    )
                    for hp in range(NPAIR):
                        nc.tensor.matmul(
                            o_ps,
                            aT_tiles[hp][:, ntl * P : (ntl + 1) * P],
                            wout_tiles[hp][:, ec * 512 : (ec + 1) * 512],
                            start=(hp == 0),
                            stop=(hp == NPAIR - 1),
                        )
                    # stage both halves of the row, DMA once: 4 KB packets
                    if ec == 0:
                        state["o_sb"] = p_ostage.tile(
                            [P, 1024], F32, tag="o_sb", name="o_sb"
                        )
                    o_sb = state["o_sb"]
                    if act_copy:
                        # tail region: copy on the (idle) ACT engine so the
                        # DVE queue carries only the final epilogue chain
                        nc.scalar.activation(
                            out=o_sb[:, ec * 512 : (ec + 1) * 512],
                            in_=o_ps,
                            func=AF.Copy,
                        )
                    else:
                        nc.vector.tensor_copy(
                            o_sb[:, ec * 512 : (ec + 1) * 512], o_ps
                        )
                    if ec == 1:
                        # spread output rows across queues so no single DMA
                        # queue's backlog serializes the kernel tail; keep the
                        # last ic OFF the gpsimd swdge queue - its slow drain
                        # otherwise ends the kernel ~5us late
                        if ic == NNC - 1:
                            q = nc.scalar if nt % 2 == 0 else nc.sync
                        elif nt % 2 == 1:
                            q = nc.gpsimd
                        else:
                            q = nc.sync
                        q.dma_start(out=out_t[nt], in_=o_sb)

                groups.append(group)
            return groups

        pending = []  # out-projection groups owed by the previous ic
        budget = [0]  # groups the current chunk's filler may still emit

        def filler(jp):
            if pending and budget[0] > 0:
                budget[0] -= 1
                pending.pop(0)()

        for ic in range(NNC):
            aT_tiles = []
            delayed = None  # (av_ps, aT) of the early chunk, epilogue owed
            for hp in range(NPAIR):
                if hp == 0 and ic == 0:
                    # defer the early chunk's epilogue until hp1's dots are
                    # queued so its serial den/recip/broadcast chain doesn't
                    # leave the PE and DVE with nothing to run
                    aT0 = p_aT.tile([P, 512], BF16, tag="aT", name="aT0")
                    delayed = (early_av, aT0)
                    aT_tiles.append(aT0)
                    continue
                av_ps = [
                    ps_av.tile([P, 512], F32, tag="av", name=f"av{s}")
                    for s in range(2)
                ]
                # on the last ic, hold back 4 of the previous ic's groups
                # to fill the dead window behind the final epilogue chain
                budget[0] = (4 if ic == NNC - 1 else NJP) if hp == 0 else 0
                emit_dots_av(hp, ic, av_ps, filler=filler if hp == 0 else None)
                if delayed is not None:
                    emit_epilogue(0, 0, delayed[0], delayed[1])
                    delayed = None
                if ic == NNC - 1 and hp == NPAIR - 1:
                    # final chunk: emit the epilogue FIRST so its den copies
                    # hit the DVE queue immediately after the last av; then
                    # fill the chain's ~7us serial latency with PE work that
                    # does not need aT[hp3]: the held-back previous-ic groups
                    # and stage 1 (hp0-2 terms) of the first four groups.
                    aT = p_aT.tile([P, 512], BF16, tag="aT", name="aT3")
                    emit_epilogue(hp, ic, av_ps, aT)
                    aT_tiles.append(aT)
                    for g, group in enumerate(list(pending)):
                        group(pool=ps_out if g % 2 == 0 else ps_mm, act_copy=True)
                    del pending[:]
                    tail_stage1 = []
                    t1pools = [(ps_out, "o"), (ps_mm, "mm"), (ps_mm, "mm"), (ps_av, "av")]
                    for g in range(4):
                        nt = 4 * ic + g // 2
                        ntl = g // 2
                        ec = g % 2
                        pl, tg = t1pools[g]
                        o_ps = pl.tile([P, 512], F32, tag=tg, name="t1")
                        for h2 in range(3):
                            nc.tensor.matmul(
                                o_ps,
                                aT_tiles[h2][:, ntl * P : (ntl + 1) * P],
                                wout_tiles[h2][:, ec * 512 : (ec + 1) * 512],
                                start=(h2 == 0),
                                stop=False,
                            )
                        tail_stage1.append((o_ps, nt, ntl, ec))
                    continue
                aT = p_aT.tile([P, 512], BF16, tag="aT", name=f"aT{hp}")
                emit_epilogue(hp, ic, av_ps, aT)
                aT_tiles.append(aT)
            leftovers = list(pending)
            del pending[:]
            if ic < NNC - 1:
                pending.extend(make_groups(aT_tiles, ic))
            # leftovers (independent PE work) land right after the last
            # epilogue's emission so they execute under its DVE/gpsimd chain;
            # rotate through the now-idle dots pool to pipeline
            for g, group in enumerate(leftovers):
                group(pool=ps_out if g % 2 == 0 else ps_mm)

        # tail stage 2: finish the four pre-accumulated groups (add hp3,
        # stage, ship), then the remaining four groups with PSUM rotation.
        tstate = {}

        def tail_ship(o_sb_ec, nt, o_ps):
            if o_sb_ec == 0:
                tstate["o_sb"] = p_ostage.tile([P, 1024], F32, tag="o_sb", name="o_sb")
            o_sb = tstate["o_sb"]
            nc.scalar.activation(
                out=o_sb[:, o_sb_ec * 512 : (o_sb_ec + 1) * 512], in_=o_ps, func=AF.Copy
            )
            if o_sb_ec == 1:
                q = nc.scalar if nt % 2 == 0 else nc.sync
                q.dma_start(out=out_t[nt], in_=o_sb)

        for o_ps, nt, ntl, ec in tail_stage1:
            nc.tensor.matmul(
                o_ps,
                aT_tiles[3][:, ntl * P : (ntl + 1) * P],
                wout_tiles[3][:, ec * 512 : (ec + 1) * 512],
                start=False,
                stop=True,
            )
            tail_ship(ec, nt, o_ps)
        for g in range(4, NJP):
            nt = 4 * (NNC - 1) + g // 2
            ntl = g // 2
            ec = g % 2
            pl, tg = (ps_out, "o") if (g // 2) % 2 == 0 else (ps_mm, "mm")
            o_ps = pl.tile([P, 512], F32, tag=tg, name="t2")
            for h2 in range(NPAIR):
                nc.tensor.matmul(
                    o_ps,
                    aT_tiles[h2][:, ntl * P : (ntl + 1) * P],
                    wout_tiles[h2][:, ec * 512 : (ec + 1) * 512],
                    start=(h2 == 0),
                    stop=(h2 == NPAIR - 1),
                )
            tail_ship(ec, nt, o_ps)

    nc.compile()
    return nc


_NC = None


def _get_program():
    global _NC
    if _NC is None:
        _NC = build_program()
    return _NC


INNER = 1024
BFD = ml_dtypes.bfloat16


def kernel(x, W_qkv, W_out, b_out):
    x = np.asarray(x, dtype=np.float32)
    W_qkv = np.asarray(W_qkv, dtype=np.float32)
    W_out = np.asarray(W_out, dtype=np.float32)
    b_out = np.asarray(b_out, dtype=np.float32)
    B = x.shape[0]

    nc = _get_program()

    def pack_w(w):  # [1024, 512] -> [128, 8*512], [p, dt, c]
        return w.reshape(NDT, P, CH).transpose(1, 0, 2)

    in_maps = []
    for b in range(B):
        # xt_p[p, pc, dt, n'] = x[b].T[dt*128+p, pc*256+n']
        xtb = (
            x[b].T.reshape(NDT, P, 8, 256).transpose(1, 2, 0, 3).reshape(P, -1)
        ).astype(BFD)
        xtb = np.ascontiguousarray(xtb)
        for hh in range(2):
            cs = hh * CH
            wq = pack_w(W_qkv[:, cs : cs + CH])
            wk = pack_w(W_qkv[:, INNER + cs : INNER + cs + CH])
            wv = pack_w(W_qkv[:, 2 * INNER + cs : 2 * INNER + cs + CH])
            wqkv_pk = np.stack([wq, wk, wv], axis=1).reshape(P, -1).astype(BFD)
            wout_pk = (
                W_out[cs : cs + CH, :].reshape(NPAIR, P, D).transpose(1, 0, 2)
            ).reshape(P, -1).astype(BFD)
            in_maps.append(
                {
                    "xt": xtb,
                    "wqkv": np.ascontiguousarray(wqkv_pk),
                    "wout": np.ascontiguousarray(wout_pk),
                }
            )
    res = run_bass_kernel_spmd(nc, in_maps, core_ids=list(range(8)))
    out = np.empty((B, NSEQ, D), dtype=np.float32)
    for b in range(B):
        out[b] = res.results[2 * b]["out"] + res.results[2 * b + 1]["out"] + b_out
    return out


# revision 62
# speedup vs baseline: 1.0206x; 1.0206x over previous
"""Trainium2 Bass kernel for nn_Attention_86698209837214.

Multi-head attention: out = softmax(q k^T / 8) v @ W_out + b_out with
B=4, N=2048, DIM=1024, H=16, Dh=64.

Sharding: 8 cores = (batch b in 0..3) x (head-half hh in 0..1); each core
computes 8 heads of one batch. Host pre-transposes x[b], slices weights,
converts everything to bf16; host sums the two per-core partial outputs
per batch and adds b_out.

All matmul operands are bf16 (PSUM accumulation fp32). bf16 halves input
DMA and - critically - enables the PE fast-weight-load path that fp32r
(FP32-HIGH mode) disables, so LDWEIGHTS hides behind matmul streaming.
Measured end-to-end rel err vs the fp32 reference: ~5e-3 (gate 2e-2).

Every matmul in the kernel is a uniform [128,128]x[128,512] bf16 shape so
the PE never switches tile config: kT is stored zero-padded per head slot
(the other head's 64 partitions zeroed; dots stream the full qT slice and
the zero weights kill the cross terms) and v slots are padded to 128 cols.
Steady-state matmul spacing measured 215 ns (213 ns streaming floor).

DMA: inputs are host-packed so per-partition rows are 4-8 KB contiguous
runs (the DMA dispatcher is packet-rate bound: 1-2 KB packets move at
~20-40 GB/s/queue vs full rate at 8 KB), split across the two HWDGE
queues (SP + ACT). Output rows are spread across SP/gpsimd/ACT queues so
no single queue's backlog serializes the kernel tail.

Device dataflow per core:
  1. v = x @ Wv into v_aug tiles ([64 v | 1 ones | 63 zero] per slot);
     the ones column makes attn@v also produce the softmax denominator
     (row 64 of the av accumulator).
  2. kT (zero-padded per head slot), qT = (x @ Wk/Wq)^T in [c, n] layout.
  3. Attention with ic (i-chunk of 512) outer, hp (head pair) inner:
     dots^T per (s, j-tile); exp on ScalarE (scale=1/8 folded, no max
     subtraction - logits ~N(0,1)) writing bf16; attn@v lags dots by one
     j-pair so the PE never waits on a fresh exp. ScalarE exp is the
     co-critical engine (~278 us busy); an early chunk (hp0, ic0) runs
     during the qT projections to start it ~30 us sooner.
     Epilogue: denominator row -> partition 0 via DMA hop, fast
     reciprocal, gpsimd partition_broadcast, DVE multiply -> aT bf16.
  4. Out-projection with K=512 accumulated over all four head pairs in
     PSUM, so the core emits one full [2048,1024] fp32 partial (host
     adds the two per-batch cores + b_out). Each ic's eight PSUM groups
     are woven into the next ic's first chunk as fillers (the single
     out-proj PSUM bank's turnaround hides behind dots/av streaming);
     the last ic holds 4 groups back to fill the final epilogue window.

Known run-to-run variance: the chip's engine clocks flip between a fast
and a ~1.2x slower DVFS state per run (matmul median 379 vs 454 ns);
measured spans 428 us (fast) / ~506 us (slow) vs 619 us baseline.
"""

import sys

for _p in ("/opt/trn_rl_repo",):
    if _p not in sys.path:
        sys.path.append(_p)

from contextlib import ExitStack

import numpy as np
import ml_dtypes

import concourse.bass as bass  # noqa: F401
import concourse.tile as tile
from concourse import bacc, mybir
from concourse.bass_utils import run_bass_kernel_spmd

F32 = mybir.dt.float32
BF16 = mybir.dt.bfloat16
AF = mybir.ActivationFunctionType

P = 128
NSEQ = 2048  # sequence length per batch
D = 1024  # model dim
CH = 512  # per-core head-dim width (8 heads x 64)
DH = 64
NPAIR = 4  # head pairs per core (c-tiles of 128)
NDT = D // P  # 8 d-tiles
NNT = NSEQ // P  # 16 n-tiles
NNC = NSEQ // 512  # 4 n-chunks
NJP = NNT // 2  # 8 j-tile pairs
SCALE = 0.125  # DIM_HEAD ** -0.5


def build_program():
    nc = bacc.Bacc("TRN2", target_bir_lowering=False, debug=False)

    # Host-packed layouts: per-partition rows are large contiguous runs so
    # DMA packets are 4-8 KB (the dispatcher is packet-rate bound; 1-2 KB
    # packets measured ~20-40 GB/s/queue vs full rate at 8 KB).
    # xt_p[p, pc, dt, n'] = x^T[dt*128+p, pc*256+n']  (8 pieces of 256)
    xt_p = nc.dram_tensor("xt", [P, 8 * NDT * 256], BF16, kind="ExternalInput")
    # wqkv_p[p, proj, dt, c'] = W_proj[dt*128+p, c']  (proj: 0=q, 1=k, 2=v)
    wqkv_p = nc.dram_tensor("wqkv", [P, 3 * NDT * 512], BF16, kind="ExternalInput")
    # wout_p[p, ct, e] = W_out[ct*128+p, e]
    wout_p = nc.dram_tensor("wout", [P, NPAIR * D], BF16, kind="ExternalInput")
    out = nc.dram_tensor("out", [NSEQ, D], F32, kind="ExternalOutput")

    out_t = out.ap().rearrange("(nt p) e -> nt p e", p=P)  # [16, 128, 1024]

    with tile.TileContext(nc) as tc, ExitStack() as ctx:
        # ---- persistent pools ----
        p_qk = ctx.enter_context(tc.tile_pool(name="p_qk", bufs=1))  # 32 KB/p
        p_v = ctx.enter_context(tc.tile_pool(name="p_v", bufs=1))  # ~16 KB/p
        p_small = ctx.enter_context(tc.tile_pool(name="p_small", bufs=1))
        # PSUM: dots 2x[128,1024] (4 banks) + av 3x[65,512] (3) + out (1) = 8
        ps_mm = ctx.enter_context(tc.tile_pool(name="ps_mm", bufs=2, space="PSUM"))
        ps_av = ctx.enter_context(tc.tile_pool(name="ps_av", bufs=3, space="PSUM"))
        ps_out = ctx.enter_context(tc.tile_pool(name="ps_out", bufs=1, space="PSUM"))

        # attention-phase persistent pools (created up front: pool pop order
        # must be LIFO w.r.t. the temporary phase-A/B pools below)
        p_wout = ctx.enter_context(tc.tile_pool(name="p_wout", bufs=1))
        p_exp = ctx.enter_context(tc.tile_pool(name="p_exp", bufs=10))  # 20 KB/p
        p_aT = ctx.enter_context(tc.tile_pool(name="p_aT", bufs=8))
        p_den = ctx.enter_context(tc.tile_pool(name="p_den", bufs=1))
        p_recip = ctx.enter_context(tc.tile_pool(name="p_recip", bufs=1))
        p_bcast = ctx.enter_context(tc.tile_pool(name="p_bcast", bufs=3))
        p_ostage = ctx.enter_context(tc.tile_pool(name="p_ostage", bufs=2))

        # dummy exp: pulls the ~2.7us ACT_TABLE_LOAD for the Exp set into the
        # initial DMA wait instead of the first real softmax tile
        warm_in = p_small.tile([P, 1], BF16, tag="warm_in")
        nc.gpsimd.memset(warm_in, 1.0)
        warm = p_small.tile([P, 1], F32, tag="warm")
        nc.scalar.activation(out=warm, in_=warm_in, func=AF.Exp, scale=1.0)

        # ---- phase A: load weights + xt; compute v_aug ----
        # Input DMAs split across the two HWDGE queues (SP + ACT; ACT is
        # otherwise idle until the early attention chunk) in arrival order
        # of first use: v needs (wv, xt nc0..3), then kT needs wk, qT wq.
        # Pool stacks pushed xt,wq,wk,wv so they pop LIFO as phases finish.
        st_xt = ExitStack()
        p_xt = st_xt.enter_context(tc.tile_pool(name="p_xt", bufs=1))  # 32 KB/p
        st_wq = ExitStack()
        p_wq = st_wq.enter_context(tc.tile_pool(name="p_wq", bufs=1))  # 8 KB/p
        st_wk = ExitStack()
        p_wk = st_wk.enter_context(tc.tile_pool(name="p_wk", bufs=1))  # 8 KB/p
        st_wv = ExitStack()
        p_wv = st_wv.enter_context(tc.tile_pool(name="p_wv", bufs=1))  # 8 KB/p

        xt_sb = p_xt.tile([P, 8 * NDT * 256], BF16, tag="xt")
        w_sbs = [
            p_wq.tile([P, NDT * 512], BF16, tag="wq", name="wq"),
            p_wk.tile([P, NDT * 512], BF16, tag="wk", name="wk"),
            p_wv.tile([P, NDT * 512], BF16, tag="wv", name="wv"),
        ]
        xt_pieces = xt_sb.rearrange("p (pc r) -> p pc r", r=NDT * 256)
        xt_view = xt_sb.rearrange("p (pc dt n) -> p pc dt n", dt=NDT, n=256)
        xt_dram = xt_p.ap().rearrange("p (pc r) -> p pc r", r=NDT * 256)
        w_dram = wqkv_p.ap().rearrange("p (pr r) -> p pr r", r=NDT * 512)

        # wv in two dt-halves so v's first accumulation isn't gated on 1 MB
        hw = NDT * 256
        nc.scalar.dma_start(out=w_sbs[2][:, 0:hw], in_=w_dram[:, 2][:, 0:hw])
        for pc in range(8):
            q = nc.sync if pc % 2 == 0 else nc.scalar
            q.dma_start(out=xt_pieces[:, pc], in_=xt_dram[:, pc])
            if pc == 0:
                nc.scalar.dma_start(
                    out=w_sbs[2][:, hw : 2 * hw], in_=w_dram[:, 2][:, hw : 2 * hw]
                )
        nc.sync.dma_start(out=w_sbs[1], in_=w_dram[:, 1])  # wk
        nc.scalar.dma_start(out=w_sbs[0], in_=w_dram[:, 0])  # wq

        wout_sb = p_wout.tile([P, NPAIR * D], BF16, tag="wout")
        nc.sync.dma_start(out=wout_sb, in_=wout_p.ap())
        wout_tiles = [wout_sb[:, ct * D : (ct + 1) * D] for ct in range(NPAIR)]

        def xt_sl(dt, n0, w):
            pc, off = divmod(n0, 256)
            if off + w <= 256:
                base = (pc * NDT + dt) * 256 + off
                return xt_sb[:, base : base + w]
            assert off == 0 and w % 256 == 0
            return xt_view[:, pc : pc + w // 256, dt, :]

        def w_sl(proj, dt, c0, w):
            base = dt * 512 + c0
            return w_sbs[proj][:, base : base + w]

        # v_aug: per head-slot sg, 128 cols = [v_sg (64) | ones (1) | 0 (63)].
        # The ones column makes attn@v also produce the softmax denominator;
        # the zero pad keeps every stationary a full 128-column weight so the
        # PE never switches tile config and FWL stays eligible.
        v_tiles = []
        for nt in range(NNT):
            dst = p_v.tile([P, 8 * P], BF16, tag=f"v{nt}")
            pad = dst.rearrange("p (h c) -> p h c", c=P)[:, :, DH:P]
            nc.gpsimd.memset(pad, 0.0)
            ones_dst = dst.rearrange("p (h c) -> p h c", c=P)[:, :, DH : DH + 1]
            nc.gpsimd.memset(ones_dst, 1.0)
            v_tiles.append(dst)
        for nt in range(NNT):
            dst = v_tiles[nt]
            acc = ps_mm.tile([P, 512], F32, tag="mm")
            for dt_i in range(NDT):
                nc.tensor.matmul(
                    acc,
                    xt_sl(dt_i, nt * P, P),
                    w_sl(2, dt_i, 0, 512),
                    start=(dt_i == 0),
                    stop=(dt_i == NDT - 1),
                )
            v_dst = dst.rearrange("p (h c) -> p h c", c=P)[:, :, 0:DH]
            nc.vector.tensor_copy(v_dst, acc.rearrange("p (h c) -> p h c", c=DH))
        st_wv.close()

        # ---- phase B: kT (zero-padded per head-slot), then qT ----
        # kT is stored per head-slot s as [128, 2048] with the other head's
        # 64 partitions zeroed, so dots matmuls are full [128,128]x[128,512]
        # (moving = the full qT slice; zero weights kill the cross terms).
        kT_pad = []
        for ct in range(NPAIR):
            pair = []
            for s in range(2):
                t = p_qk.tile([P, NSEQ], BF16, tag=f"kp{ct}{s}", name=f"kp{ct}{s}")
                z0, z1 = (DH, P) if s == 0 else (0, DH)
                # gpsimd, not DVE: DVE must keep pace with the qk PSUM copies
                nc.gpsimd.memset(t[z0:z1, :], 0.0)
                pair.append(t)
            kT_pad.append(pair)
        qT_tiles = []

        def emit_qk_tile(which, proj, ct):
            woff = ct * P
            if which == "q":
                dst = p_qk.tile([P, NSEQ], BF16, tag=f"qT{ct}", name=f"qT{ct}")
            for nch in range(NNC):
                acc = ps_mm.tile([P, 512], F32, tag="mm", name="acc")
                for dt_i in range(NDT):
                    nc.tensor.matmul(
                        acc,
                        w_sl(proj, dt_i, woff, P),
                        xt_sl(dt_i, nch * 512, 512),
                        start=(dt_i == 0),
                        stop=(dt_i == NDT - 1),
                    )
                sl = slice(nch * 512, (nch + 1) * 512)
                if which == "k":
                    nc.vector.tensor_copy(kT_pad[ct][0][0:DH, sl], acc[0:DH, :])
                    nc.vector.tensor_copy(kT_pad[ct][1][DH:P, sl], acc[DH:P, :])
                else:
                    nc.vector.tensor_copy(dst[:, sl], acc)
            if which == "q":
                qT_tiles.append(dst)

        for ct in range(NPAIR):
            emit_qk_tile("k", 1, ct)
        st_wk.close()
        emit_qk_tile("q", 0, 0)

        # per (hp, ic) attention body -------------------------------------
        def emit_dots_av(hp, ic, av_ps, filler=None):
            """dots + exp + attn@v for one (head pair, i-chunk).

            filler(jp), if given, is invoked once per j-pair to weave in
            independent PE work (the previous chunk's out-projection) so
            PSUM-bank turnarounds hide behind dots/av streaming.
            """
            i0 = ic * 512

            def emit_av(jp, exp_pair):
                for s in range(2):
                    sg = hp * 2 + s
                    for half in range(2):
                        jtx = 2 * jp + half
                        nc.tensor.matmul(
                            av_ps[s],
                            v_tiles[jtx][:, sg * P : (sg + 1) * P],
                            exp_pair[s][:, half * 512 : (half + 1) * 512],
                            start=(jp == 0 and half == 0),
                            stop=(jp == NJP - 1 and half == 1),
                        )

            prev_exp = None
            for jp in range(NJP):
                # dots for 2 j-tiles x 2 head-slots; stationary = zero-padded
                # per-head kT block, moving = full qT slice
                dots_t = [
                    ps_mm.tile([P, 1024], F32, tag="mm", name=f"dots{s}")
                    for s in range(2)
                ]
                for half in range(2):
                    jtx = 2 * jp + half
                    for s in range(2):
                        nc.tensor.matmul(
                            dots_t[s][:, half * 512 : (half + 1) * 512],
                            kT_pad[hp][s][:, jtx * P : (jtx + 1) * P],
                            qT_tiles[hp][:, i0 : i0 + 512],
                            start=True,
                            stop=True,
                        )
                if prev_exp is not None:
                    emit_av(jp - 1, prev_exp)
                if filler is not None:
                    filler(jp)
                exp_tiles = []
                for s in range(2):
                    e = p_exp.tile([P, 1024], BF16, tag="exp")
                    nc.scalar.activation(
                        out=e, in_=dots_t[s], func=AF.Exp, scale=SCALE
                    )
                    exp_tiles.append(e)
                prev_exp = exp_tiles
            emit_av(NJP - 1, prev_exp)

        def emit_epilogue(hp, ic, av_ps, aT):
            """normalize: rows 0:64 = unnormalized attn-out, row 64 = denom."""
            den_hi = p_den.tile([65, 1024], F32, tag="den_hi")
            for s in range(2):
                nc.vector.tensor_copy(
                    den_hi[64:65, s * 512 : (s + 1) * 512], av_ps[s][64:65, :]
                )
            den_sb = p_den.tile([1, 1024], F32, tag="den_sb")
            nc.gpsimd.dma_start(out=den_sb, in_=den_hi[64:65, :])
            recip = p_recip.tile([1, 1024], F32, tag="recip")
            nc.vector.reciprocal_approx_fast(out=recip, in_=den_sb)
            bcast = []
            for s in range(2):
                bc = p_bcast.tile([DH, 512], F32, tag="bcast", name=f"bc{s}")
                nc.gpsimd.partition_broadcast(
                    out_ap=bc, in_ap=recip[:, s * 512 : (s + 1) * 512]
                )
                bcast.append(bc)
            nc.vector.tensor_mul(aT[0:DH, :], av_ps[0][0:DH, :], bcast[0])
            tmp = p_bcast.tile([DH, 512], BF16, tag="tmp")
            nc.vector.tensor_mul(tmp, av_ps[1][0:DH, :], bcast[1])
            nc.gpsimd.dma_start(out=aT[DH:P, :], in_=tmp)

        # ---- early chunk (hp=0, ic=0): ScalarE gets exp work while the
        # remaining qT tiles occupy the PE. Epilogue deferred to main loop.
        # early_av[0] lives in the (still idle) out-proj bank so the main
        # loop's first av pair doesn't WAR-wait on this chunk's epilogue
        early_av = [
            ps_out.tile([P, 512], F32, tag="o", name="eav0"),
            ps_av.tile([P, 512], F32, tag="av", name="eav1"),
        ]
        emit_dots_av(0, 0, early_av)

        for ct in range(1, NPAIR):
            emit_qk_tile("q", 0, ct)
        st_wq.close()
        st_xt.close()

        # ---- main loop: ic outer, hp inner; out-projection per ic with
        # K=512 accumulated over all four head pairs. Each ic's eight
        # out-projection groups are spread as fillers over ALL FOUR of the
        # next ic's chunks (2 per chunk) so per-chunk PE work stays balanced
        # against the fixed 16 exps/chunk on ScalarE, and the single
        # out-proj PSUM bank's turnaround hides behind dots/av streaming.
        def make_groups(aT_tiles, ic):
            state = {}
            groups = []
            for g in range(NJP):
                nt = 4 * ic + g // 2
                ntl = g // 2
                ec = g % 2

                def group(pool=None, act_copy=False, nt=nt, ntl=ntl, ec=ec):
                    pl = pool if pool is not None else ps_out
                    o_ps = pl.tile(
                        [P, 512], F32, tag="o" if pl is ps_out else "mm", name="ops"
                    )
                    for hp in range(NPAIR):
                        nc.tensor.matmul(
                            o_ps,
                            aT_tiles[hp][:, ntl * P : (ntl + 1) * P],
                            wout_tiles[hp][:, ec * 512 : (ec + 1) * 512],
                            start=(hp == 0),
                            stop=(hp == NPAIR - 1),
                        )
                    # stage both halves of the row, DMA once: 4 KB packets
                    if ec == 0:
                        state["o_sb"] = p_ostage.tile(
                            [P, 1024], F32, tag="o_sb", name="o_sb"
                        )
                    o_sb = state["o_sb"]
                    if act_copy:
                        # tail region: copy on the (idle) ACT engine so the
                        # DVE queue carries only the final epilogue chain
                        nc.scalar.activation(
                            out=o_sb[:, ec * 512 : (ec + 1) * 512],
                            in_=o_ps,
                            func=AF.Copy,
                        )
                    else:
                        nc.vector.tensor_copy(
                            o_sb[:, ec * 512 : (ec + 1) * 512], o_ps
                        )
                    if ec == 1:
                        # spread output rows across queues so no single DMA
                        # queue's backlog serializes the kernel tail; keep the
                        # last ic OFF the gpsimd swdge queue - its slow drain
                        # otherwise ends the kernel ~5us late
                        if ic == NNC - 1:
                            q = nc.scalar if nt % 2 == 0 else nc.sync
                        elif nt % 2 == 1:
                            q = nc.gpsimd
                        else:
                            q = nc.sync
                        q.dma_start(out=out_t[nt], in_=o_sb)

                groups.append(group)
            return groups

        pending = []  # out-projection groups owed by the previous ic
        budget = [0]  # groups the current chunk's filler may still emit

        def filler(jp):
            if jp in (2, 5) and pending and budget[0] > 0:
                budget[0] -= 1
                pending.pop(0)()

        for ic in range(NNC):
            aT_tiles = []
            delayed = None  # (av_ps, aT) of the early chunk, epilogue owed
            for hp in range(NPAIR):
                if hp == 0 and ic == 0:
                    # defer the early chunk's epilogue until hp1's dots are
                    # queued so its serial den/recip/broadcast chain doesn't
                    # leave the PE and DVE with nothing to run
                    aT0 = p_aT.tile([P, 512], BF16, tag="aT", name="aT0")
                    delayed = (early_av, aT0)
                    aT_tiles.append(aT0)
                    continue
                av_ps = [
                    ps_av.tile([P, 512], F32, tag="av", name=f"av{s}")
                    for s in range(2)
                ]
                # spread the previous ic's out-proj groups 2-per-chunk over
                # all four chunks so per-chunk PE work stays balanced against
                # ScalarE's fixed 16 exps/chunk (hp0-only fillers made those
                # chunks PE-heavy and idled ACT ~3us at each boundary). The
                # very last chunk keeps 0 so its leftovers fill the final
                # epilogue window instead.
                budget[0] = 0 if (ic == NNC - 1 and hp == NPAIR - 1) else 2
                emit_dots_av(hp, ic, av_ps, filler=filler)
                if delayed is not None:
                    emit_epilogue(0, 0, delayed[0], delayed[1])
                    delayed = None
                if ic == NNC - 1 and hp == NPAIR - 1:
                    # final chunk: emit the epilogue FIRST so its den copies
                    # hit the DVE queue immediately after the last av; then
                    # fill the chain's ~7us serial latency with PE work that
                    # does not need aT[hp3]: the held-back previous-ic groups
                    # and stage 1 (hp0-2 terms) of the first four groups.
                    aT = p_aT.tile([P, 512], BF16, tag="aT", name="aT3")
                    emit_epilogue(hp, ic, av_ps, aT)
                    aT_tiles.append(aT)
                    for g, group in enumerate(list(pending)):
                        group(pool=ps_out if g % 2 == 0 else ps_mm, act_copy=True)
                    del pending[:]
                    tail_stage1 = []
                    t1pools = [(ps_out, "o"), (ps_mm, "mm"), (ps_mm, "mm"), (ps_av, "av")]
                    for g in range(4):
                        nt = 4 * ic + g // 2
                        ntl = g // 2
                        ec = g % 2
                        pl, tg = t1pools[g]
                        o_ps = pl.tile([P, 512], F32, tag=tg, name="t1")
                        for h2 in range(3):
                            nc.tensor.matmul(
                                o_ps,
                                aT_tiles[h2][:, ntl * P : (ntl + 1) * P],
                                wout_tiles[h2][:, ec * 512 : (ec + 1) * 512],
                                start=(h2 == 0),
                                stop=False,
                            )
                        tail_stage1.append((o_ps, nt, ntl, ec))
                    continue
                aT = p_aT.tile([P, 512], BF16, tag="aT", name=f"aT{hp}")
                emit_epilogue(hp, ic, av_ps, aT)
                aT_tiles.append(aT)
            leftovers = list(pending)
            del pending[:]
            if ic < NNC - 1:
                pending.extend(make_groups(aT_tiles, ic))
            # leftovers (independent PE work) land right after the last
            # epilogue's emission so they execute under its DVE/gpsimd chain;
            # rotate through the now-idle dots pool to pipeline
            for g, group in enumerate(leftovers):
                group(pool=ps_out if g % 2 == 0 else ps_mm)

        # tail stage 2: finish the four pre-accumulated groups (add hp3,
        # stage, ship), then the remaining four groups with PSUM rotation.
        tstate = {}

        def tail_ship(o_sb_ec, nt, o_ps):
            if o_sb_ec == 0:
                tstate["o_sb"] = p_ostage.tile([P, 1024], F32, tag="o_sb", name="o_sb")
            o_sb = tstate["o_sb"]
            nc.scalar.activation(
                out=o_sb[:, o_sb_ec * 512 : (o_sb_ec + 1) * 512], in_=o_ps, func=AF.Copy
            )
            if o_sb_ec == 1:
                q = nc.scalar if nt % 2 == 0 else nc.sync
                q.dma_start(out=out_t[nt], in_=o_sb)

        for o_ps, nt, ntl, ec in tail_stage1:
            nc.tensor.matmul(
                o_ps,
                aT_tiles[3][:, ntl * P : (ntl + 1) * P],
                wout_tiles[3][:, ec * 512 : (ec + 1) * 512],
                start=False,
                stop=True,
            )
            tail_ship(ec, nt, o_ps)
        for g in range(4, NJP):
            nt = 4 * (NNC - 1) + g // 2
            ntl = g // 2
            ec = g % 2
            pl, tg = (ps_out, "o") if (g // 2) % 2 == 0 else (ps_mm, "mm")
            o_ps = pl.tile([P, 512], F32, tag=tg, name="t2")
            for h2 in range(NPAIR):
                nc.tensor.matmul(
                    o_ps,
                    aT_tiles[h2][:, ntl * P : (ntl + 1) * P],
                    wout_tiles[h2][:, ec * 512 : (ec + 1) * 512],
                    start=(h2 == 0),
                    stop=(h2 == NPAIR - 1),
                )
            tail_ship(ec, nt, o_ps)

    nc.compile()
    return nc


_NC = None


def _get_program():
    global _NC
    if _NC is None:
        _NC = build_program()
    return _NC


INNER = 1024
BFD = ml_dtypes.bfloat16


def kernel(x, W_qkv, W_out, b_out):
    x = np.asarray(x, dtype=np.float32)
    W_qkv = np.asarray(W_qkv, dtype=np.float32)
    W_out = np.asarray(W_out, dtype=np.float32)
    b_out = np.asarray(b_out, dtype=np.float32)
    B = x.shape[0]

    nc = _get_program()

    def pack_w(w):  # [1024, 512] -> [128, 8*512], [p, dt, c]
        return w.reshape(NDT, P, CH).transpose(1, 0, 2)

    in_maps = []
    for b in range(B):
        # xt_p[p, pc, dt, n'] = x[b].T[dt*128+p, pc*256+n']
        xtb = (
            x[b].T.reshape(NDT, P, 8, 256).transpose(1, 2, 0, 3).reshape(P, -1)
        ).astype(BFD)
        xtb = np.ascontiguousarray(xtb)
        for hh in range(2):
            cs = hh * CH
            wq = pack_w(W_qkv[:, cs : cs + CH])
            wk = pack_w(W_qkv[:, INNER + cs : INNER + cs + CH])
            wv = pack_w(W_qkv[:, 2 * INNER + cs : 2 * INNER + cs + CH])
            wqkv_pk = np.stack([wq, wk, wv], axis=1).reshape(P, -1).astype(BFD)
            wout_pk = (
                W_out[cs : cs + CH, :].reshape(NPAIR, P, D).transpose(1, 0, 2)
            ).reshape(P, -1).astype(BFD)
            in_maps.append(
                {
                    "xt": xtb,
                    "wqkv": np.ascontiguousarray(wqkv_pk),
                    "wout": np.ascontiguousarray(wout_pk),
                }
            )
    res = run_bass_kernel_spmd(nc, in_maps, core_ids=list(range(8)))
    out = np.empty((B, NSEQ, D), dtype=np.float32)
    for b in range(B):
        out[b] = res.results[2 * b]["out"] + res.results[2 * b + 1]["out"] + b_out
    return out


# revision 65
# speedup vs baseline: 1.2114x; 1.1869x over previous
"""Trainium2 Bass kernel for nn_Attention_86698209837214.

Multi-head attention: out = softmax(q k^T / 8) v @ W_out + b_out with
B=4, N=2048, DIM=1024, H=16, Dh=64.

Sharding: 8 cores = (batch b in 0..3) x (head-half hh in 0..1); each core
computes 8 heads of one batch. Host pre-transposes x[b], slices weights,
converts everything to bf16; host sums the two per-core partial outputs
per batch and adds b_out.

All matmul operands are bf16 (PSUM accumulation fp32). bf16 halves input
DMA and - critically - enables the PE fast-weight-load path that fp32r
(FP32-HIGH mode) disables, so LDWEIGHTS hides behind matmul streaming.
Measured end-to-end rel err vs the fp32 reference: ~5e-3 (gate 2e-2).

Every matmul in the kernel is a uniform [128,128]x[128,512] bf16 shape so
the PE never switches tile config: kT is stored zero-padded per head slot
(the other head's 64 partitions zeroed; dots stream the full qT slice and
the zero weights kill the cross terms) and v slots are padded to 128 cols.
Steady-state matmul spacing measured 215 ns (213 ns streaming floor).

DMA: inputs are host-packed so per-partition rows are 4-8 KB contiguous
runs (the DMA dispatcher is packet-rate bound: 1-2 KB packets move at
~20-40 GB/s/queue vs full rate at 8 KB), split across the two HWDGE
queues (SP + ACT). Output rows are spread across SP/gpsimd/ACT queues so
no single queue's backlog serializes the kernel tail.

Device dataflow per core:
  1. v = x @ Wv into v_aug tiles ([64 v | 1 ones | 63 zero] per slot);
     the ones column makes attn@v also produce the softmax denominator
     (row 64 of the av accumulator).
  2. kT (zero-padded per head slot), qT = (x @ Wk/Wq)^T in [c, n] layout.
  3. Attention with ic (i-chunk of 512) outer, hp (head pair) inner:
     dots^T per (s, j-tile); exp on ScalarE (scale=1/8 folded, no max
     subtraction - logits ~N(0,1)) writing bf16; attn@v lags dots by one
     j-pair so the PE never waits on a fresh exp. ScalarE exp is the
     co-critical engine (~278 us busy); an early chunk (hp0, ic0) runs
     during the qT projections to start it ~30 us sooner.
     Epilogue: denominator row -> partition 0 via DMA hop, fast
     reciprocal, gpsimd partition_broadcast, DVE multiply -> aT bf16.
  4. Out-projection with K=512 accumulated over all four head pairs in
     PSUM, so the core emits one full [2048,1024] fp32 partial (host
     adds the two per-batch cores + b_out). Each ic's eight PSUM groups
     are woven into the next ic's first chunk as fillers (the single
     out-proj PSUM bank's turnaround hides behind dots/av streaming);
     the last ic holds 4 groups back to fill the final epilogue window.

Known run-to-run variance: the chip's engine clocks flip between a fast
and a ~1.2x slower DVFS state per run (matmul median 379 vs 454 ns);
measured spans 428 us (fast) / ~506 us (slow) vs 619 us baseline.
"""

import sys

for _p in ("/opt/trn_rl_repo",):
    if _p not in sys.path:
        sys.path.append(_p)

from contextlib import ExitStack

import numpy as np
import ml_dtypes

import concourse.bass as bass  # noqa: F401
import concourse.tile as tile
from concourse import bacc, mybir
from concourse.bass_utils import run_bass_kernel_spmd

F32 = mybir.dt.float32
BF16 = mybir.dt.bfloat16
AF = mybir.ActivationFunctionType

P = 128
NSEQ = 2048  # sequence length per batch
D = 1024  # model dim
CH = 512  # per-core head-dim width (8 heads x 64)
DH = 64
NPAIR = 4  # head pairs per core (c-tiles of 128)
NDT = D // P  # 8 d-tiles
NNT = NSEQ // P  # 16 n-tiles
NNC = NSEQ // 512  # 4 n-chunks
NJP = NNT // 2  # 8 j-tile pairs
SCALE = 0.125  # DIM_HEAD ** -0.5


def build_program():
    nc = bacc.Bacc("TRN2", target_bir_lowering=False, debug=False)

    # Host-packed layouts: per-partition rows are large contiguous runs so
    # DMA packets are 4-8 KB (the dispatcher is packet-rate bound; 1-2 KB
    # packets measured ~20-40 GB/s/queue vs full rate at 8 KB).
    # xt_p[p, pc, dt, n'] = x^T[dt*128+p, pc*256+n']  (8 pieces of 256)
    xt_p = nc.dram_tensor("xt", [P, 8 * NDT * 256], BF16, kind="ExternalInput")
    # wqkv_p[p, proj, dt, c'] = W_proj[dt*128+p, c']  (proj: 0=q, 1=k, 2=v)
    wqkv_p = nc.dram_tensor("wqkv", [P, 3 * NDT * 512], BF16, kind="ExternalInput")
    # wout_p[p, ct, e] = W_out[ct*128+p, e]
    wout_p = nc.dram_tensor("wout", [P, NPAIR * D], BF16, kind="ExternalInput")
    out = nc.dram_tensor("out", [NSEQ, D], F32, kind="ExternalOutput")

    out_t = out.ap().rearrange("(nt p) e -> nt p e", p=P)  # [16, 128, 1024]

    with tile.TileContext(nc) as tc, ExitStack() as ctx:
        # ---- persistent pools ----
        p_qk = ctx.enter_context(tc.tile_pool(name="p_qk", bufs=1))  # 32 KB/p
        p_v = ctx.enter_context(tc.tile_pool(name="p_v", bufs=1))  # ~16 KB/p
        p_small = ctx.enter_context(tc.tile_pool(name="p_small", bufs=1))
        # PSUM: dots 2x[128,1024] (4 banks) + av 3x[65,512] (3) + out (1) = 8
        ps_mm = ctx.enter_context(tc.tile_pool(name="ps_mm", bufs=2, space="PSUM"))
        ps_av = ctx.enter_context(tc.tile_pool(name="ps_av", bufs=3, space="PSUM"))
        ps_out = ctx.enter_context(tc.tile_pool(name="ps_out", bufs=1, space="PSUM"))

        # attention-phase persistent pools (created up front: pool pop order
        # must be LIFO w.r.t. the temporary phase-A/B pools below)
        p_wout = ctx.enter_context(tc.tile_pool(name="p_wout", bufs=1))
        p_exp = ctx.enter_context(tc.tile_pool(name="p_exp", bufs=10))  # 20 KB/p
        p_aT = ctx.enter_context(tc.tile_pool(name="p_aT", bufs=8))
        p_den = ctx.enter_context(tc.tile_pool(name="p_den", bufs=1))
        p_recip = ctx.enter_context(tc.tile_pool(name="p_recip", bufs=1))
        p_bcast = ctx.enter_context(tc.tile_pool(name="p_bcast", bufs=3))
        p_ostage = ctx.enter_context(tc.tile_pool(name="p_ostage", bufs=2))

        # dummy exp: pulls the ~2.7us ACT_TABLE_LOAD for the Exp set into the
        # initial DMA wait instead of the first real softmax tile
        warm_in = p_small.tile([P, 1], BF16, tag="warm_in")
        nc.gpsimd.memset(warm_in, 1.0)
        warm = p_small.tile([P, 1], F32, tag="warm")
        nc.scalar.activation(out=warm, in_=warm_in, func=AF.Exp, scale=1.0)

        # ---- phase A: load weights + xt; compute v_aug ----
        # Input DMAs split across the two HWDGE queues (SP + ACT; ACT is
        # otherwise idle until the early attention chunk) in arrival order
        # of first use: v needs (wv, xt nc0..3), then kT needs wk, qT wq.
        # Pool stacks pushed xt,wq,wk,wv so they pop LIFO as phases finish.
        st_xt = ExitStack()
        p_xt = st_xt.enter_context(tc.tile_pool(name="p_xt", bufs=1))  # 32 KB/p
        st_wq = ExitStack()
        p_wq = st_wq.enter_context(tc.tile_pool(name="p_wq", bufs=1))  # 8 KB/p
        st_wk = ExitStack()
        p_wk = st_wk.enter_context(tc.tile_pool(name="p_wk", bufs=1))  # 8 KB/p
        st_wv = ExitStack()
        p_wv = st_wv.enter_context(tc.tile_pool(name="p_wv", bufs=1))  # 8 KB/p

        xt_sb = p_xt.tile([P, 8 * NDT * 256], BF16, tag="xt")
        w_sbs = [
            p_wq.tile([P, NDT * 512], BF16, tag="wq", name="wq"),
            p_wk.tile([P, NDT * 512], BF16, tag="wk", name="wk"),
            p_wv.tile([P, NDT * 512], BF16, tag="wv", name="wv"),
        ]
        xt_pieces = xt_sb.rearrange("p (pc r) -> p pc r", r=NDT * 256)
        xt_view = xt_sb.rearrange("p (pc dt n) -> p pc dt n", dt=NDT, n=256)
        xt_dram = xt_p.ap().rearrange("p (pc r) -> p pc r", r=NDT * 256)
        w_dram = wqkv_p.ap().rearrange("p (pr r) -> p pr r", r=NDT * 512)

        # wv in two dt-halves so v's first accumulation isn't gated on 1 MB
        hw = NDT * 256
        nc.scalar.dma_start(out=w_sbs[2][:, 0:hw], in_=w_dram[:, 2][:, 0:hw])
        for pc in range(8):
            q = nc.sync if pc % 2 == 0 else nc.scalar
            q.dma_start(out=xt_pieces[:, pc], in_=xt_dram[:, pc])
            if pc == 0:
                nc.scalar.dma_start(
                    out=w_sbs[2][:, hw : 2 * hw], in_=w_dram[:, 2][:, hw : 2 * hw]
                )
        nc.sync.dma_start(out=w_sbs[1], in_=w_dram[:, 1])  # wk
        nc.scalar.dma_start(out=w_sbs[0], in_=w_dram[:, 0])  # wq

        wout_sb = p_wout.tile([P, NPAIR * D], BF16, tag="wout")
        nc.sync.dma_start(out=wout_sb, in_=wout_p.ap())
        wout_tiles = [wout_sb[:, ct * D : (ct + 1) * D] for ct in range(NPAIR)]

        def xt_sl(dt, n0, w):
            pc, off = divmod(n0, 256)
            if off + w <= 256:
                base = (pc * NDT + dt) * 256 + off
                return xt_sb[:, base : base + w]
            assert off == 0 and w % 256 == 0
            return xt_view[:, pc : pc + w // 256, dt, :]

        def w_sl(proj, dt, c0, w):
            base = dt * 512 + c0
            return w_sbs[proj][:, base : base + w]

        # v_aug: per head-slot sg, 128 cols = [v_sg (64) | ones (1) | 0 (63)].
        # The ones column makes attn@v also produce the softmax denominator;
        # the zero pad keeps every stationary a full 128-column weight so the
        # PE never switches tile config and FWL stays eligible.
        v_tiles = []
        for nt in range(NNT):
            dst = p_v.tile([P, 8 * P], BF16, tag=f"v{nt}")
            pad = dst.rearrange("p (h c) -> p h c", c=P)[:, :, DH:P]
            nc.gpsimd.memset(pad, 0.0)
            ones_dst = dst.rearrange("p (h c) -> p h c", c=P)[:, :, DH : DH + 1]
            nc.gpsimd.memset(ones_dst, 1.0)
            v_tiles.append(dst)
        for nt in range(NNT):
            dst = v_tiles[nt]
            acc = ps_mm.tile([P, 512], F32, tag="mm")
            for dt_i in range(NDT):
                nc.tensor.matmul(
                    acc,
                    xt_sl(dt_i, nt * P, P),
                    w_sl(2, dt_i, 0, 512),
                    start=(dt_i == 0),
                    stop=(dt_i == NDT - 1),
                )
            v_dst = dst.rearrange("p (h c) -> p h c", c=P)[:, :, 0:DH]
            nc.vector.tensor_copy(v_dst, acc.rearrange("p (h c) -> p h c", c=DH))
        st_wv.close()

        # ---- phase B: kT (zero-padded per head-slot), then qT ----
        # kT is stored per head-slot s as [128, 2048] with the other head's
        # 64 partitions zeroed, so dots matmuls are full [128,128]x[128,512]
        # (moving = the full qT slice; zero weights kill the cross terms).
        kT_pad = []
        for ct in range(NPAIR):
            pair = []
            for s in range(2):
                t = p_qk.tile([P, NSEQ], BF16, tag=f"kp{ct}{s}", name=f"kp{ct}{s}")
                z0, z1 = (DH, P) if s == 0 else (0, DH)
                # gpsimd, not DVE: DVE must keep pace with the qk PSUM copies
                nc.gpsimd.memset(t[z0:z1, :], 0.0)
                pair.append(t)
            kT_pad.append(pair)
        qT_tiles = []

        def emit_qk_tile(which, proj, ct):
            woff = ct * P
            if which == "q":
                dst = p_qk.tile([P, NSEQ], BF16, tag=f"qT{ct}", name=f"qT{ct}")
            for nch in range(NNC):
                acc = ps_mm.tile([P, 512], F32, tag="mm", name="acc")
                for dt_i in range(NDT):
                    nc.tensor.matmul(
                        acc,
                        w_sl(proj, dt_i, woff, P),
                        xt_sl(dt_i, nch * 512, 512),
                        start=(dt_i == 0),
                        stop=(dt_i == NDT - 1),
                    )
                sl = slice(nch * 512, (nch + 1) * 512)
                if which == "k":
                    nc.vector.tensor_copy(kT_pad[ct][0][0:DH, sl], acc[0:DH, :])
                    nc.vector.tensor_copy(kT_pad[ct][1][DH:P, sl], acc[DH:P, :])
                else:
                    nc.vector.tensor_copy(dst[:, sl], acc)
            if which == "q":
                qT_tiles.append(dst)

        for ct in range(NPAIR):
            emit_qk_tile("k", 1, ct)
        st_wk.close()
        emit_qk_tile("q", 0, 0)

        # per (hp, ic) attention body -------------------------------------
        def emit_dots_av(hp, ic, av_ps, filler=None):
            """dots + exp + attn@v for one (head pair, i-chunk).

            filler(jp), if given, is invoked once per j-pair to weave in
            independent PE work (the previous chunk's out-projection) so
            PSUM-bank turnarounds hide behind dots/av streaming.
            """
            i0 = ic * 512

            def emit_av(jp, exp_pair):
                for s in range(2):
                    sg = hp * 2 + s
                    for half in range(2):
                        jtx = 2 * jp + half
                        nc.tensor.matmul(
                            av_ps[s],
                            v_tiles[jtx][:, sg * P : (sg + 1) * P],
                            exp_pair[s][:, half * 512 : (half + 1) * 512],
                            start=(jp == 0 and half == 0),
                            stop=(jp == NJP - 1 and half == 1),
                        )

            prev_exp = None
            for jp in range(NJP):
                # dots for 2 j-tiles x 2 head-slots; stationary = zero-padded
                # per-head kT block, moving = full qT slice
                dots_t = [
                    ps_mm.tile([P, 1024], F32, tag="mm", name=f"dots{s}")
                    for s in range(2)
                ]
                for half in range(2):
                    jtx = 2 * jp + half
                    for s in range(2):
                        nc.tensor.matmul(
                            dots_t[s][:, half * 512 : (half + 1) * 512],
                            kT_pad[hp][s][:, jtx * P : (jtx + 1) * P],
                            qT_tiles[hp][:, i0 : i0 + 512],
                            start=True,
                            stop=True,
                        )
                if prev_exp is not None:
                    emit_av(jp - 1, prev_exp)
                if filler is not None:
                    filler(jp)
                exp_tiles = []
                for s in range(2):
                    e = p_exp.tile([P, 1024], BF16, tag="exp")
                    nc.scalar.activation(
                        out=e, in_=dots_t[s], func=AF.Exp, scale=SCALE
                    )
                    exp_tiles.append(e)
                prev_exp = exp_tiles
            emit_av(NJP - 1, prev_exp)

        def emit_epilogue(hp, ic, av_ps, aT, tail=False):
            """normalize: rows 0:64 = unnormalized attn-out, row 64 = denom.

            tail=True (final chunk): the cross-partition DMA hops and one
            broadcast ride the idle SP queue instead of gpsimd, so this
            epilogue's serial chain doesn't also queue behind the previous
            epilogue's gpsimd hops.
            """
            dma_q = nc.sync if tail else nc.gpsimd
            den_hi = p_den.tile([65, 1024], F32, tag="den_hi")
            for s in range(2):
                nc.vector.tensor_copy(
                    den_hi[64:65, s * 512 : (s + 1) * 512], av_ps[s][64:65, :]
                )
            den_sb = p_den.tile([1, 1024], F32, tag="den_sb")
            dma_q.dma_start(out=den_sb, in_=den_hi[64:65, :])
            recip = p_recip.tile([1, 1024], F32, tag="recip")
            nc.vector.reciprocal_approx_fast(out=recip, in_=den_sb)
            bcast = []
            for s in range(2):
                bc = p_bcast.tile([DH, 512], F32, tag="bcast", name=f"bc{s}")
                nc.gpsimd.partition_broadcast(
                    out_ap=bc, in_ap=recip[:, s * 512 : (s + 1) * 512]
                )
                bcast.append(bc)
            nc.vector.tensor_mul(aT[0:DH, :], av_ps[0][0:DH, :], bcast[0])
            tmp = p_bcast.tile([DH, 512], BF16, tag="tmp")
            nc.vector.tensor_mul(tmp, av_ps[1][0:DH, :], bcast[1])
            dma_q.dma_start(out=aT[DH:P, :], in_=tmp)

        # ---- early chunk (hp=0, ic=0): ScalarE gets exp work while the
        # remaining qT tiles occupy the PE. Epilogue deferred to main loop.
        # early_av[0] lives in the (still idle) out-proj bank so the main
        # loop's first av pair doesn't WAR-wait on this chunk's epilogue
        early_av = [
            ps_out.tile([P, 512], F32, tag="o", name="eav0"),
            ps_av.tile([P, 512], F32, tag="av", name="eav1"),
        ]
        emit_dots_av(0, 0, early_av)

        for ct in range(1, NPAIR):
            emit_qk_tile("q", 0, ct)
        st_wq.close()
        st_xt.close()

        # ---- main loop: ic outer, hp inner; out-projection per ic with
        # K=512 accumulated over all four head pairs. Each ic's eight
        # out-projection groups are spread as fillers over ALL FOUR of the
        # next ic's chunks (2 per chunk) so per-chunk PE work stays balanced
        # against the fixed 16 exps/chunk on ScalarE, and the single
        # out-proj PSUM bank's turnaround hides behind dots/av streaming.
        def make_groups(aT_tiles, ic):
            state = {}
            groups = []
            for g in range(NJP):
                nt = 4 * ic + g // 2
                ntl = g // 2
                ec = g % 2

                def group(pool=None, act_copy=False, nt=nt, ntl=ntl, ec=ec):
                    pl = pool if pool is not None else ps_out
                    o_ps = pl.tile(
                        [P, 512], F32, tag="o" if pl is ps_out else "mm", name="ops"
                    )
                    for hp in range(NPAIR):
                        nc.tensor.matmul(
                            o_ps,
                            aT_tiles[hp][:, ntl * P : (ntl + 1) * P],
                            wout_tiles[hp][:, ec * 512 : (ec + 1) * 512],
                            start=(hp == 0),
                            stop=(hp == NPAIR - 1),
                        )
                    # stage both halves of the row, DMA once: 4 KB packets
                    if ec == 0:
                        state["o_sb"] = p_ostage.tile(
                            [P, 1024], F32, tag="o_sb", name="o_sb"
                        )
                    o_sb = state["o_sb"]
                    if act_copy:
                        # tail region: copy on the (idle) ACT engine so the
                        # DVE queue carries only the final epilogue chain
                        nc.scalar.activation(
                            out=o_sb[:, ec * 512 : (ec + 1) * 512],
                            in_=o_ps,
                            func=AF.Copy,
                        )
                    else:
                        nc.vector.tensor_copy(
                            o_sb[:, ec * 512 : (ec + 1) * 512], o_ps
                        )
                    if ec == 1:
                        # spread output rows across queues so no single DMA
                        # queue's backlog serializes the kernel tail; keep the
                        # last ic OFF the gpsimd swdge queue - its slow drain
                        # otherwise ends the kernel ~5us late
                        if ic == NNC - 1:
                            q = nc.scalar if nt % 2 == 0 else nc.sync
                        elif nt % 2 == 1:
                            q = nc.gpsimd
                        else:
                            q = nc.sync
                        q.dma_start(out=out_t[nt], in_=o_sb)

                groups.append(group)
            return groups

        pending = []  # out-projection groups owed by the previous ic
        budget = [0]  # groups the current chunk's filler may still emit

        def filler(jp):
            if jp in (2, 5) and pending and budget[0] > 0:
                budget[0] -= 1
                pending.pop(0)()

        for ic in range(NNC):
            aT_tiles = []
            delayed = None  # (av_ps, aT) of the early chunk, epilogue owed
            for hp in range(NPAIR):
                if hp == 0 and ic == 0:
                    # defer the early chunk's epilogue until hp1's dots are
                    # queued so its serial den/recip/broadcast chain doesn't
                    # leave the PE and DVE with nothing to run
                    aT0 = p_aT.tile([P, 512], BF16, tag="aT", name="aT0")
                    delayed = (early_av, aT0)
                    aT_tiles.append(aT0)
                    continue
                av_ps = [
                    ps_av.tile([P, 512], F32, tag="av", name=f"av{s}")
                    for s in range(2)
                ]
                # spread the previous ic's out-proj groups 2-per-chunk over
                # all four chunks so per-chunk PE work stays balanced against
                # ScalarE's fixed 16 exps/chunk (hp0-only fillers made those
                # chunks PE-heavy and idled ACT ~3us at each boundary). The
                # very last chunk keeps 0 so its leftovers fill the final
                # epilogue window instead.
                budget[0] = 0 if (ic == NNC - 1 and hp == NPAIR - 1) else 2
                emit_dots_av(hp, ic, av_ps, filler=filler)
                if delayed is not None:
                    emit_epilogue(0, 0, delayed[0], delayed[1])
                    delayed = None
                if ic == NNC - 1 and hp == NPAIR - 1:
                    # final chunk: emit the epilogue FIRST so its den copies
                    # hit the DVE queue immediately after the last av; then
                    # fill the chain's ~7us serial latency with PE work that
                    # does not need aT[hp3]: the held-back previous-ic groups
                    # and stage 1 (hp0-2 terms) of the first four groups.
                    aT = p_aT.tile([P, 512], BF16, tag="aT", name="aT3")
                    emit_epilogue(hp, ic, av_ps, aT, tail=True)
                    aT_tiles.append(aT)
                    for g, group in enumerate(list(pending)):
                        group(pool=ps_out if g % 2 == 0 else ps_mm, act_copy=True)
                    del pending[:]
                    tail_stage1 = []
                    t1pools = [(ps_out, "o"), (ps_mm, "mm"), (ps_mm, "mm"), (ps_av, "av")]
                    for g in range(4):
                        nt = 4 * ic + g // 2
                        ntl = g // 2
                        ec = g % 2
                        pl, tg = t1pools[g]
                        o_ps = pl.tile([P, 512], F32, tag=tg, name="t1")
                        for h2 in range(3):
                            nc.tensor.matmul(
                                o_ps,
                                aT_tiles[h2][:, ntl * P : (ntl + 1) * P],
                                wout_tiles[h2][:, ec * 512 : (ec + 1) * 512],
                                start=(h2 == 0),
                                stop=False,
                            )
                        tail_stage1.append((o_ps, nt, ntl, ec))
                    continue
                aT = p_aT.tile([P, 512], BF16, tag="aT", name=f"aT{hp}")
                emit_epilogue(hp, ic, av_ps, aT)
                aT_tiles.append(aT)
            leftovers = list(pending)
            del pending[:]
            if ic < NNC - 1:
                pending.extend(make_groups(aT_tiles, ic))
            # leftovers (independent PE work) land right after the last
            # epilogue's emission so they execute under its DVE/gpsimd chain;
            # rotate through the now-idle dots pool to pipeline
            for g, group in enumerate(leftovers):
                group(pool=ps_out if g % 2 == 0 else ps_mm)

        # tail stage 2: finish the four pre-accumulated groups (add hp3,
        # stage, ship), then the remaining four groups with PSUM rotation.
        tstate = {}

        def tail_ship(o_sb_ec, nt, o_ps):
            if o_sb_ec == 0:
                tstate["o_sb"] = p_ostage.tile([P, 1024], F32, tag="o_sb", name="o_sb")
            o_sb = tstate["o_sb"]
            nc.scalar.activation(
                out=o_sb[:, o_sb_ec * 512 : (o_sb_ec + 1) * 512], in_=o_ps, func=AF.Copy
            )
            if o_sb_ec == 1:
                q = nc.scalar if nt % 2 == 0 else nc.sync
                q.dma_start(out=out_t[nt], in_=o_sb)

        for o_ps, nt, ntl, ec in tail_stage1:
            nc.tensor.matmul(
                o_ps,
                aT_tiles[3][:, ntl * P : (ntl + 1) * P],
                wout_tiles[3][:, ec * 512 : (ec + 1) * 512],
                start=False,
                stop=True,
            )
            tail_ship(ec, nt, o_ps)
        for g in range(4, NJP):
            nt = 4 * (NNC - 1) + g // 2
            ntl = g // 2
            ec = g % 2
            pl, tg = (ps_out, "o") if (g // 2) % 2 == 0 else (ps_mm, "mm")
            o_ps = pl.tile([P, 512], F32, tag=tg, name="t2")
            for h2 in range(NPAIR):
                nc.tensor.matmul(
                    o_ps,
                    aT_tiles[h2][:, ntl * P : (ntl + 1) * P],
                    wout_tiles[h2][:, ec * 512 : (ec + 1) * 512],
                    start=(h2 == 0),
                    stop=(h2 == NPAIR - 1),
                )
            tail_ship(ec, nt, o_ps)

    nc.compile()
    return nc


_NC = None


def _get_program():
    global _NC
    if _NC is None:
        _NC = build_program()
    return _NC


INNER = 1024
BFD = ml_dtypes.bfloat16


def kernel(x, W_qkv, W_out, b_out):
    x = np.asarray(x, dtype=np.float32)
    W_qkv = np.asarray(W_qkv, dtype=np.float32)
    W_out = np.asarray(W_out, dtype=np.float32)
    b_out = np.asarray(b_out, dtype=np.float32)
    B = x.shape[0]

    nc = _get_program()

    def pack_w(w):  # [1024, 512] -> [128, 8*512], [p, dt, c]
        return w.reshape(NDT, P, CH).transpose(1, 0, 2)

    in_maps = []
    for b in range(B):
        # xt_p[p, pc, dt, n'] = x[b].T[dt*128+p, pc*256+n']
        xtb = (
            x[b].T.reshape(NDT, P, 8, 256).transpose(1, 2, 0, 3).reshape(P, -1)
        ).astype(BFD)
        xtb = np.ascontiguousarray(xtb)
        for hh in range(2):
            cs = hh * CH
            wq = pack_w(W_qkv[:, cs : cs + CH])
            wk = pack_w(W_qkv[:, INNER + cs : INNER + cs + CH])
            wv = pack_w(W_qkv[:, 2 * INNER + cs : 2 * INNER + cs + CH])
            wqkv_pk = np.stack([wq, wk, wv], axis=1).reshape(P, -1).astype(BFD)
            wout_pk = (
                W_out[cs : cs + CH, :].reshape(NPAIR, P, D).transpose(1, 0, 2)
            ).reshape(P, -1).astype(BFD)
            in_maps.append(
                {
                    "xt": xtb,
                    "wqkv": np.ascontiguousarray(wqkv_pk),
                    "wout": np.ascontiguousarray(wout_pk),
                }
            )
    res = run_bass_kernel_spmd(nc, in_maps, core_ids=list(range(8)))
    out = np.empty((B, NSEQ, D), dtype=np.float32)
    for b in range(B):
        out[b] = res.results[2 * b]["out"] + res.results[2 * b + 1]["out"] + b_out
    return out


# revision 67
# speedup vs baseline: 1.2139x; 1.0021x over previous
"""Trainium2 Bass kernel for nn_Attention_86698209837214.

Multi-head attention: out = softmax(q k^T / 8) v @ W_out + b_out with
B=4, N=2048, DIM=1024, H=16, Dh=64.

Sharding: 8 cores = (batch b in 0..3) x (head-half hh in 0..1); each core
computes 8 heads of one batch. Host pre-transposes x[b], slices weights,
converts everything to bf16; host sums the two per-core partial outputs
per batch and adds b_out.

All matmul operands are bf16 (PSUM accumulation fp32). bf16 halves input
DMA and - critically - enables the PE fast-weight-load path that fp32r
(FP32-HIGH mode) disables, so LDWEIGHTS hides behind matmul streaming.
Measured end-to-end rel err vs the fp32 reference: ~5e-3 (gate 2e-2).

Every matmul in the kernel is a uniform [128,128]x[128,512] bf16 shape so
the PE never switches tile config: kT is stored zero-padded per head slot
(the other head's 64 partitions zeroed; dots stream the full qT slice and
the zero weights kill the cross terms) and v slots are padded to 128 cols.
Steady-state matmul spacing measured 215 ns (213 ns streaming floor).

DMA: inputs are host-packed so per-partition rows are 4-8 KB contiguous
runs (the DMA dispatcher is packet-rate bound: 1-2 KB packets move at
~20-40 GB/s/queue vs full rate at 8 KB), split across the two HWDGE
queues (SP + ACT). Output rows are spread across SP/gpsimd/ACT queues so
no single queue's backlog serializes the kernel tail.

Device dataflow per core:
  1. v = x @ Wv into v_aug tiles ([64 v | 1 ones | 63 zero] per slot);
     the ones column makes attn@v also produce the softmax denominator
     (row 64 of the av accumulator).
  2. kT (zero-padded per head slot), qT = (x @ Wk/Wq)^T in [c, n] layout.
  3. Attention with ic (i-chunk of 512) outer, hp (head pair) inner:
     dots^T per (s, j-tile); exp on ScalarE (scale=1/8 folded, no max
     subtraction - logits ~N(0,1)) writing bf16; attn@v lags dots by one
     j-pair so the PE never waits on a fresh exp. ScalarE exp is the
     co-critical engine (~278 us busy); an early chunk (hp0, ic0) runs
     during the qT projections to start it ~30 us sooner.
     Epilogue: denominator row -> partition 0 via DMA hop, fast
     reciprocal, gpsimd partition_broadcast, DVE multiply -> aT bf16.
  4. Out-projection with K=512 accumulated over all four head pairs in
     PSUM, so the core emits one full [2048,1024] fp32 partial (host
     adds the two per-batch cores + b_out). Each ic's eight PSUM groups
     are spread 2-per-chunk over the next ic's four chunks as fillers,
     keeping per-chunk PE work balanced against ScalarE's fixed 16
     exps/chunk. The final chunk instead emits its epilogue first (DMA
     hops on the idle SP queue), fills the chain's serial latency with
     the held-back groups plus stage-1 (hp0-2) of its own out-projection,
     and finishes with the hp3 terms once aT[hp3] lands.

Known run-to-run variance: the chip's engine clocks flip between a fast
and a ~1.2x slower DVFS state per run (matmul median 379 vs 454 ns);
measured spans 415 us (fast) / ~493 us (slow) vs 619 us baseline.
"""

import sys

for _p in ("/opt/trn_rl_repo",):
    if _p not in sys.path:
        sys.path.append(_p)

from contextlib import ExitStack

import numpy as np
import ml_dtypes

import concourse.bass as bass  # noqa: F401
import concourse.tile as tile
from concourse import bacc, mybir
from concourse.bass_utils import run_bass_kernel_spmd

F32 = mybir.dt.float32
BF16 = mybir.dt.bfloat16
AF = mybir.ActivationFunctionType

P = 128
NSEQ = 2048  # sequence length per batch
D = 1024  # model dim
CH = 512  # per-core head-dim width (8 heads x 64)
DH = 64
NPAIR = 4  # head pairs per core (c-tiles of 128)
NDT = D // P  # 8 d-tiles
NNT = NSEQ // P  # 16 n-tiles
NNC = NSEQ // 512  # 4 n-chunks
NJP = NNT // 2  # 8 j-tile pairs
SCALE = 0.125  # DIM_HEAD ** -0.5


def build_program():
    nc = bacc.Bacc("TRN2", target_bir_lowering=False, debug=False)

    # Host-packed layouts: per-partition rows are large contiguous runs so
    # DMA packets are 4-8 KB (the dispatcher is packet-rate bound; 1-2 KB
    # packets measured ~20-40 GB/s/queue vs full rate at 8 KB).
    # xt_p[p, pc, dt, n'] = x^T[dt*128+p, pc*256+n']  (8 pieces of 256)
    xt_p = nc.dram_tensor("xt", [P, 8 * NDT * 256], BF16, kind="ExternalInput")
    # wqkv_p[p, proj, dt, c'] = W_proj[dt*128+p, c']  (proj: 0=q, 1=k, 2=v)
    wqkv_p = nc.dram_tensor("wqkv", [P, 3 * NDT * 512], BF16, kind="ExternalInput")
    # wout_p[p, ct, e] = W_out[ct*128+p, e]
    wout_p = nc.dram_tensor("wout", [P, NPAIR * D], BF16, kind="ExternalInput")
    out = nc.dram_tensor("out", [NSEQ, D], BF16, kind="ExternalOutput")

    out_t = out.ap().rearrange("(nt p) e -> nt p e", p=P)  # [16, 128, 1024]

    with tile.TileContext(nc) as tc, ExitStack() as ctx:
        # ---- persistent pools ----
        p_qk = ctx.enter_context(tc.tile_pool(name="p_qk", bufs=1))  # 32 KB/p
        p_v = ctx.enter_context(tc.tile_pool(name="p_v", bufs=1))  # ~16 KB/p
        p_small = ctx.enter_context(tc.tile_pool(name="p_small", bufs=1))
        # PSUM: dots 2x[128,1024] (4 banks) + av 3x[65,512] (3) + out (1) = 8
        ps_mm = ctx.enter_context(tc.tile_pool(name="ps_mm", bufs=2, space="PSUM"))
        ps_av = ctx.enter_context(tc.tile_pool(name="ps_av", bufs=3, space="PSUM"))
        ps_out = ctx.enter_context(tc.tile_pool(name="ps_out", bufs=1, space="PSUM"))

        # attention-phase persistent pools (created up front: pool pop order
        # must be LIFO w.r.t. the temporary phase-A/B pools below)
        p_wout = ctx.enter_context(tc.tile_pool(name="p_wout", bufs=1))
        p_exp = ctx.enter_context(tc.tile_pool(name="p_exp", bufs=10))  # 20 KB/p
        p_aT = ctx.enter_context(tc.tile_pool(name="p_aT", bufs=8))
        p_den = ctx.enter_context(tc.tile_pool(name="p_den", bufs=1))
        p_recip = ctx.enter_context(tc.tile_pool(name="p_recip", bufs=1))
        p_bcast = ctx.enter_context(tc.tile_pool(name="p_bcast", bufs=3))
        p_ostage = ctx.enter_context(tc.tile_pool(name="p_ostage", bufs=2))

        # dummy exp: pulls the ~2.7us ACT_TABLE_LOAD for the Exp set into the
        # initial DMA wait instead of the first real softmax tile
        warm_in = p_small.tile([P, 1], BF16, tag="warm_in")
        nc.gpsimd.memset(warm_in, 1.0)
        warm = p_small.tile([P, 1], F32, tag="warm")
        nc.scalar.activation(out=warm, in_=warm_in, func=AF.Exp, scale=1.0)

        # ---- phase A: load weights + xt; compute v_aug ----
        # Input DMAs split across the two HWDGE queues (SP + ACT; ACT is
        # otherwise idle until the early attention chunk) in arrival order
        # of first use: v needs (wv, xt nc0..3), then kT needs wk, qT wq.
        # Pool stacks pushed xt,wq,wk,wv so they pop LIFO as phases finish.
        st_xt = ExitStack()
        p_xt = st_xt.enter_context(tc.tile_pool(name="p_xt", bufs=1))  # 32 KB/p
        st_wq = ExitStack()
        p_wq = st_wq.enter_context(tc.tile_pool(name="p_wq", bufs=1))  # 8 KB/p
        st_wk = ExitStack()
        p_wk = st_wk.enter_context(tc.tile_pool(name="p_wk", bufs=1))  # 8 KB/p
        st_wv = ExitStack()
        p_wv = st_wv.enter_context(tc.tile_pool(name="p_wv", bufs=1))  # 8 KB/p

        xt_sb = p_xt.tile([P, 8 * NDT * 256], BF16, tag="xt")
        w_sbs = [
            p_wq.tile([P, NDT * 512], BF16, tag="wq", name="wq"),
            p_wk.tile([P, NDT * 512], BF16, tag="wk", name="wk"),
            p_wv.tile([P, NDT * 512], BF16, tag="wv", name="wv"),
        ]
        xt_pieces = xt_sb.rearrange("p (pc r) -> p pc r", r=NDT * 256)
        xt_view = xt_sb.rearrange("p (pc dt n) -> p pc dt n", dt=NDT, n=256)
        xt_dram = xt_p.ap().rearrange("p (pc r) -> p pc r", r=NDT * 256)
        w_dram = wqkv_p.ap().rearrange("p (pr r) -> p pr r", r=NDT * 512)

        # wv in two dt-halves so v's first accumulation isn't gated on 1 MB
        hw = NDT * 256
        nc.scalar.dma_start(out=w_sbs[2][:, 0:hw], in_=w_dram[:, 2][:, 0:hw])
        for pc in range(8):
            q = nc.sync if pc % 2 == 0 else nc.scalar
            q.dma_start(out=xt_pieces[:, pc], in_=xt_dram[:, pc])
            if pc == 0:
                nc.scalar.dma_start(
                    out=w_sbs[2][:, hw : 2 * hw], in_=w_dram[:, 2][:, hw : 2 * hw]
                )
        nc.sync.dma_start(out=w_sbs[1], in_=w_dram[:, 1])  # wk
        nc.scalar.dma_start(out=w_sbs[0], in_=w_dram[:, 0])  # wq

        wout_sb = p_wout.tile([P, NPAIR * D], BF16, tag="wout")
        nc.sync.dma_start(out=wout_sb, in_=wout_p.ap())
        wout_tiles = [wout_sb[:, ct * D : (ct + 1) * D] for ct in range(NPAIR)]

        def xt_sl(dt, n0, w):
            pc, off = divmod(n0, 256)
            if off + w <= 256:
                base = (pc * NDT + dt) * 256 + off
                return xt_sb[:, base : base + w]
            assert off == 0 and w % 256 == 0
            return xt_view[:, pc : pc + w // 256, dt, :]

        def w_sl(proj, dt, c0, w):
            base = dt * 512 + c0
            return w_sbs[proj][:, base : base + w]

        # v_aug: per head-slot sg, 128 cols = [v_sg (64) | ones (1) | 0 (63)].
        # The ones column makes attn@v also produce the softmax denominator;
        # the zero pad keeps every stationary a full 128-column weight so the
        # PE never switches tile config and FWL stays eligible.
        v_tiles = []
        for nt in range(NNT):
            dst = p_v.tile([P, 8 * P], BF16, tag=f"v{nt}")
            pad = dst.rearrange("p (h c) -> p h c", c=P)[:, :, DH:P]
            nc.gpsimd.memset(pad, 0.0)
            ones_dst = dst.rearrange("p (h c) -> p h c", c=P)[:, :, DH : DH + 1]
            nc.gpsimd.memset(ones_dst, 1.0)
            v_tiles.append(dst)
        for nt in range(NNT):
            dst = v_tiles[nt]
            acc = ps_mm.tile([P, 512], F32, tag="mm")
            for dt_i in range(NDT):
                nc.tensor.matmul(
                    acc,
                    xt_sl(dt_i, nt * P, P),
                    w_sl(2, dt_i, 0, 512),
                    start=(dt_i == 0),
                    stop=(dt_i == NDT - 1),
                )
            v_dst = dst.rearrange("p (h c) -> p h c", c=P)[:, :, 0:DH]
            nc.vector.tensor_copy(v_dst, acc.rearrange("p (h c) -> p h c", c=DH))
        st_wv.close()

        # ---- phase B: kT (zero-padded per head-slot), then qT ----
        # kT is stored per head-slot s as [128, 2048] with the other head's
        # 64 partitions zeroed, so dots matmuls are full [128,128]x[128,512]
        # (moving = the full qT slice; zero weights kill the cross terms).
        kT_pad = []
        for ct in range(NPAIR):
            pair = []
            for s in range(2):
                t = p_qk.tile([P, NSEQ], BF16, tag=f"kp{ct}{s}", name=f"kp{ct}{s}")
                z0, z1 = (DH, P) if s == 0 else (0, DH)
                # gpsimd, not DVE: DVE must keep pace with the qk PSUM copies
                nc.gpsimd.memset(t[z0:z1, :], 0.0)
                pair.append(t)
            kT_pad.append(pair)
        qT_tiles = []

        def emit_qk_tile(which, proj, ct):
            woff = ct * P
            if which == "q":
                dst = p_qk.tile([P, NSEQ], BF16, tag=f"qT{ct}", name=f"qT{ct}")
            for nch in range(NNC):
                acc = ps_mm.tile([P, 512], F32, tag="mm", name="acc")
                for dt_i in range(NDT):
                    nc.tensor.matmul(
                        acc,
                        w_sl(proj, dt_i, woff, P),
                        xt_sl(dt_i, nch * 512, 512),
                        start=(dt_i == 0),
                        stop=(dt_i == NDT - 1),
                    )
                sl = slice(nch * 512, (nch + 1) * 512)
                if which == "k":
                    nc.vector.tensor_copy(kT_pad[ct][0][0:DH, sl], acc[0:DH, :])
                    nc.vector.tensor_copy(kT_pad[ct][1][DH:P, sl], acc[DH:P, :])
                else:
                    nc.vector.tensor_copy(dst[:, sl], acc)
            if which == "q":
                qT_tiles.append(dst)

        for ct in range(NPAIR):
            emit_qk_tile("k", 1, ct)
        st_wk.close()
        emit_qk_tile("q", 0, 0)

        # per (hp, ic) attention body -------------------------------------
        def emit_dots_av(hp, ic, av_ps, filler=None):
            """dots + exp + attn@v for one (head pair, i-chunk).

            filler(jp), if given, is invoked once per j-pair to weave in
            independent PE work (the previous chunk's out-projection) so
            PSUM-bank turnarounds hide behind dots/av streaming.
            """
            i0 = ic * 512

            def emit_av(jp, exp_pair):
                for s in range(2):
                    sg = hp * 2 + s
                    for half in range(2):
                        jtx = 2 * jp + half
                        nc.tensor.matmul(
                            av_ps[s],
                            v_tiles[jtx][:, sg * P : (sg + 1) * P],
                            exp_pair[s][:, half * 512 : (half + 1) * 512],
                            start=(jp == 0 and half == 0),
                            stop=(jp == NJP - 1 and half == 1),
                        )

            prev_exp = None
            for jp in range(NJP):
                # dots for 2 j-tiles x 2 head-slots; stationary = zero-padded
                # per-head kT block, moving = full qT slice
                dots_t = [
                    ps_mm.tile([P, 1024], F32, tag="mm", name=f"dots{s}")
                    for s in range(2)
                ]
                for half in range(2):
                    jtx = 2 * jp + half
                    for s in range(2):
                        nc.tensor.matmul(
                            dots_t[s][:, half * 512 : (half + 1) * 512],
                            kT_pad[hp][s][:, jtx * P : (jtx + 1) * P],
                            qT_tiles[hp][:, i0 : i0 + 512],
                            start=True,
                            stop=True,
                        )
                if prev_exp is not None:
                    emit_av(jp - 1, prev_exp)
                if filler is not None:
                    filler(jp)
                exp_tiles = []
                for s in range(2):
                    e = p_exp.tile([P, 1024], BF16, tag="exp")
                    nc.scalar.activation(
                        out=e, in_=dots_t[s], func=AF.Exp, scale=SCALE
                    )
                    exp_tiles.append(e)
                prev_exp = exp_tiles
            emit_av(NJP - 1, prev_exp)

        def emit_epilogue(hp, ic, av_ps, aT, tail=False):
            """normalize: rows 0:64 = unnormalized attn-out, row 64 = denom.

            tail=True (final chunk): the cross-partition DMA hops and one
            broadcast ride the idle SP queue instead of gpsimd, so this
            epilogue's serial chain doesn't also queue behind the previous
            epilogue's gpsimd hops.
            """
            dma_q = nc.sync if tail else nc.gpsimd
            den_hi = p_den.tile([65, 1024], F32, tag="den_hi")
            for s in range(2):
                nc.vector.tensor_copy(
                    den_hi[64:65, s * 512 : (s + 1) * 512], av_ps[s][64:65, :]
                )
            den_sb = p_den.tile([1, 1024], F32, tag="den_sb")
            dma_q.dma_start(out=den_sb, in_=den_hi[64:65, :])
            recip = p_recip.tile([1, 1024], F32, tag="recip")
            nc.vector.reciprocal_approx_fast(out=recip, in_=den_sb)
            bcast = []
            for s in range(2):
                bc = p_bcast.tile([DH, 512], F32, tag="bcast", name=f"bc{s}")
                nc.gpsimd.partition_broadcast(
                    out_ap=bc, in_ap=recip[:, s * 512 : (s + 1) * 512]
                )
                bcast.append(bc)
            nc.vector.tensor_mul(aT[0:DH, :], av_ps[0][0:DH, :], bcast[0])
            tmp = p_bcast.tile([DH, 512], BF16, tag="tmp")
            nc.vector.tensor_mul(tmp, av_ps[1][0:DH, :], bcast[1])
            dma_q.dma_start(out=aT[DH:P, :], in_=tmp)

        # ---- early chunk (hp=0, ic=0): ScalarE gets exp work while the
        # remaining qT tiles occupy the PE. Epilogue deferred to main loop.
        # early_av[0] lives in the (still idle) out-proj bank so the main
        # loop's first av pair doesn't WAR-wait on this chunk's epilogue
        early_av = [
            ps_out.tile([P, 512], F32, tag="o", name="eav0"),
            ps_av.tile([P, 512], F32, tag="av", name="eav1"),
        ]
        emit_dots_av(0, 0, early_av)

        for ct in range(1, NPAIR):
            emit_qk_tile("q", 0, ct)
        st_wq.close()
        st_xt.close()

        # ---- main loop: ic outer, hp inner; out-projection per ic with
        # K=512 accumulated over all four head pairs. Each ic's eight
        # out-projection groups are spread as fillers over ALL FOUR of the
        # next ic's chunks (2 per chunk) so per-chunk PE work stays balanced
        # against the fixed 16 exps/chunk on ScalarE, and the single
        # out-proj PSUM bank's turnaround hides behind dots/av streaming.
        def make_groups(aT_tiles, ic):
            state = {}
            groups = []
            for g in range(NJP):
                nt = 4 * ic + g // 2
                ntl = g // 2
                ec = g % 2

                def group(pool=None, act_copy=False, nt=nt, ntl=ntl, ec=ec):
                    pl = pool if pool is not None else ps_out
                    o_ps = pl.tile(
                        [P, 512], F32, tag="o" if pl is ps_out else "mm", name="ops"
                    )
                    for hp in range(NPAIR):
                        nc.tensor.matmul(
                            o_ps,
                            aT_tiles[hp][:, ntl * P : (ntl + 1) * P],
                            wout_tiles[hp][:, ec * 512 : (ec + 1) * 512],
                            start=(hp == 0),
                            stop=(hp == NPAIR - 1),
                        )
                    # stage both halves of the row, DMA once: 4 KB packets
                    if ec == 0:
                        state["o_sb"] = p_ostage.tile(
                            [P, 1024], BF16, tag="o_sb", name="o_sb"
                        )
                    o_sb = state["o_sb"]
                    if act_copy:
                        # tail region: copy on the (idle) ACT engine so the
                        # DVE queue carries only the final epilogue chain
                        nc.scalar.activation(
                            out=o_sb[:, ec * 512 : (ec + 1) * 512],
                            in_=o_ps,
                            func=AF.Copy,
                        )
                    else:
                        nc.vector.tensor_copy(
                            o_sb[:, ec * 512 : (ec + 1) * 512], o_ps
                        )
                    if ec == 1:
                        # spread output rows across queues so no single DMA
                        # queue's backlog serializes the kernel tail; keep the
                        # last ic OFF the gpsimd swdge queue - its slow drain
                        # otherwise ends the kernel ~5us late
                        if ic == NNC - 1:
                            q = nc.scalar if nt % 2 == 0 else nc.sync
                        elif nt % 2 == 1:
                            q = nc.gpsimd
                        else:
                            q = nc.sync
                        q.dma_start(out=out_t[nt], in_=o_sb)

                groups.append(group)
            return groups

        pending = []  # out-projection groups owed by the previous ic
        budget = [0]  # groups the current chunk's filler may still emit

        def filler(jp):
            if jp in (2, 5) and pending and budget[0] > 0:
                budget[0] -= 1
                pending.pop(0)()

        for ic in range(NNC):
            aT_tiles = []
            delayed = None  # (av_ps, aT) of the early chunk, epilogue owed
            for hp in range(NPAIR):
                if hp == 0 and ic == 0:
                    # defer the early chunk's epilogue until hp1's dots are
                    # queued so its serial den/recip/broadcast chain doesn't
                    # leave the PE and DVE with nothing to run
                    aT0 = p_aT.tile([P, 512], BF16, tag="aT", name="aT0")
                    delayed = (early_av, aT0)
                    aT_tiles.append(aT0)
                    continue
                av_ps = [
                    ps_av.tile([P, 512], F32, tag="av", name=f"av{s}")
                    for s in range(2)
                ]
                # spread the previous ic's out-proj groups 2-per-chunk over
                # all four chunks so per-chunk PE work stays balanced against
                # ScalarE's fixed 16 exps/chunk (hp0-only fillers made those
                # chunks PE-heavy and idled ACT ~3us at each boundary). The
                # very last chunk keeps 0 so its leftovers fill the final
                # epilogue window instead.
                budget[0] = 0 if (ic == NNC - 1 and hp == NPAIR - 1) else 2
                emit_dots_av(hp, ic, av_ps, filler=filler)
                if delayed is not None:
                    emit_epilogue(0, 0, delayed[0], delayed[1])
                    delayed = None
                if ic == NNC - 1 and hp == NPAIR - 1:
                    # final chunk: emit the epilogue FIRST so its den copies
                    # hit the DVE queue immediately after the last av; then
                    # fill the chain's ~7us serial latency with PE work that
                    # does not need aT[hp3]: the held-back previous-ic groups
                    # and stage 1 (hp0-2 terms) of the first four groups.
                    aT = p_aT.tile([P, 512], BF16, tag="aT", name="aT3")
                    emit_epilogue(hp, ic, av_ps, aT, tail=True)
                    aT_tiles.append(aT)
                    for g, group in enumerate(list(pending)):
                        group(pool=ps_out if g % 2 == 0 else ps_mm, act_copy=True)
                    del pending[:]
                    tail_stage1 = []
                    t1pools = [(ps_out, "o"), (ps_mm, "mm"), (ps_mm, "mm"), (ps_av, "av")]
                    for g in range(4):
                        nt = 4 * ic + g // 2
                        ntl = g // 2
                        ec = g % 2
                        pl, tg = t1pools[g]
                        o_ps = pl.tile([P, 512], F32, tag=tg, name="t1")
                        for h2 in range(3):
                            nc.tensor.matmul(
                                o_ps,
                                aT_tiles[h2][:, ntl * P : (ntl + 1) * P],
                                wout_tiles[h2][:, ec * 512 : (ec + 1) * 512],
                                start=(h2 == 0),
                                stop=False,
                            )
                        tail_stage1.append((o_ps, nt, ntl, ec))
                    continue
                aT = p_aT.tile([P, 512], BF16, tag="aT", name=f"aT{hp}")
                emit_epilogue(hp, ic, av_ps, aT)
                aT_tiles.append(aT)
            leftovers = list(pending)
            del pending[:]
            if ic < NNC - 1:
                pending.extend(make_groups(aT_tiles, ic))
            # leftovers (independent PE work) land right after the last
            # epilogue's emission so they execute under its DVE/gpsimd chain;
            # rotate through the now-idle dots pool to pipeline
            for g, group in enumerate(leftovers):
                group(pool=ps_out if g % 2 == 0 else ps_mm)

        # tail stage 2: finish the four pre-accumulated groups (add hp3,
        # stage, ship), then the remaining four groups with PSUM rotation.
        tstate = {}

        def tail_ship(o_sb_ec, nt, o_ps):
            if o_sb_ec == 0:
                tstate["o_sb"] = p_ostage.tile([P, 1024], BF16, tag="o_sb", name="o_sb")
            o_sb = tstate["o_sb"]
            nc.scalar.activation(
                out=o_sb[:, o_sb_ec * 512 : (o_sb_ec + 1) * 512], in_=o_ps, func=AF.Copy
            )
            if o_sb_ec == 1:
                q = nc.scalar if nt % 2 == 0 else nc.sync
                q.dma_start(out=out_t[nt], in_=o_sb)

        for o_ps, nt, ntl, ec in tail_stage1:
            nc.tensor.matmul(
                o_ps,
                aT_tiles[3][:, ntl * P : (ntl + 1) * P],
                wout_tiles[3][:, ec * 512 : (ec + 1) * 512],
                start=False,
                stop=True,
            )
            tail_ship(ec, nt, o_ps)
        for g in range(4, NJP):
            nt = 4 * (NNC - 1) + g // 2
            ntl = g // 2
            ec = g % 2
            pl, tg = (ps_out, "o") if (g // 2) % 2 == 0 else (ps_mm, "mm")
            o_ps = pl.tile([P, 512], F32, tag=tg, name="t2")
            for h2 in range(NPAIR):
                nc.tensor.matmul(
                    o_ps,
                    aT_tiles[h2][:, ntl * P : (ntl + 1) * P],
                    wout_tiles[h2][:, ec * 512 : (ec + 1) * 512],
                    start=(h2 == 0),
                    stop=(h2 == NPAIR - 1),
                )
            tail_ship(ec, nt, o_ps)

    nc.compile()
    return nc


_NC = None


def _get_program():
    global _NC
    if _NC is None:
        _NC = build_program()
    return _NC


INNER = 1024
BFD = ml_dtypes.bfloat16


def kernel(x, W_qkv, W_out, b_out):
    x = np.asarray(x, dtype=np.float32)
    W_qkv = np.asarray(W_qkv, dtype=np.float32)
    W_out = np.asarray(W_out, dtype=np.float32)
    b_out = np.asarray(b_out, dtype=np.float32)
    B = x.shape[0]

    nc = _get_program()

    def pack_w(w):  # [1024, 512] -> [128, 8*512], [p, dt, c]
        return w.reshape(NDT, P, CH).transpose(1, 0, 2)

    in_maps = []
    for b in range(B):
        # xt_p[p, pc, dt, n'] = x[b].T[dt*128+p, pc*256+n']
        xtb = (
            x[b].T.reshape(NDT, P, 8, 256).transpose(1, 2, 0, 3).reshape(P, -1)
        ).astype(BFD)
        xtb = np.ascontiguousarray(xtb)
        for hh in range(2):
            cs = hh * CH
            wq = pack_w(W_qkv[:, cs : cs + CH])
            wk = pack_w(W_qkv[:, INNER + cs : INNER + cs + CH])
            wv = pack_w(W_qkv[:, 2 * INNER + cs : 2 * INNER + cs + CH])
            wqkv_pk = np.stack([wq, wk, wv], axis=1).reshape(P, -1).astype(BFD)
            wout_pk = (
                W_out[cs : cs + CH, :].reshape(NPAIR, P, D).transpose(1, 0, 2)
            ).reshape(P, -1).astype(BFD)
            in_maps.append(
                {
                    "xt": xtb,
                    "wqkv": np.ascontiguousarray(wqkv_pk),
                    "wout": np.ascontiguousarray(wout_pk),
                }
            )
    res = run_bass_kernel_spmd(nc, in_maps, core_ids=list(range(8)))
    out = np.empty((B, NSEQ, D), dtype=np.float32)
    for b in range(B):
        out[b] = (
            res.results[2 * b]["out"].astype(np.float32)
            + res.results[2 * b + 1]["out"].astype(np.float32)
            + b_out
        )
    return out
